# revision 6
# baseline (speedup 1.0000x reference)
"""Trainium2 Bass kernel for nn_LocalWLGNN (gnn_message_passing).

Reference computation (per layer l, x: [N, D]):
    out = (1+eps) * x
    for hop in range(H):
        agg = segment_sum(x[scatter_idx[hop]], node_idx[hop], N)
        out += relu((x + agg) @ w1[l,hop]) @ w2[l,hop]
    x = out

Sharding: 8 cores, core k owns destination nodes [k*N/8, (k+1)*N/8).
Each (core, hop) edge list is split by source-node window (lo/hi, so row
indices fit the int16 dma_gather contract), and within each pass the
destinations are sorted by in-degree so that "round j = j-th in-edge of
every destination" covers a contiguous position prefix.  On device:
  - dma_gather (SWDGE) fetches rows from DRAM in large merged chunks
    (rounds concatenated, split only at the chunk capacity),
  - DVE accumulates round segments into a per-pass aggregate (pass order),
  - dma_scatter_add (SBUF parity-split CCE mode) permutes+adds both pass
    aggregates back into canonical order on top of x_k,
  - PE transposes to feature-major, runs the 2-layer MLP,
  - hop outputs accumulate into a feature-major f32 accumulator,
  - an 8-core bf16 AllGather (Shared-output fast path) republishes the
    new node features between layers.
The x-path (features, gather, aggregate, MLP operands) runs in bfloat16;
hop-sum accumulation and the final output stay float32.
"""

import numpy as np


# ---------------------------------------------------------------- config

def make_cfg(N, D, E, H, L, ncores, wlo, cap):
    nsh = N // ncores
    nshp = -(-nsh // 128) * 128          # padded positions per core
    sidxn = -(-nsh // 16) * 16           # scatter num_idxs (16-mult)
    assert nshp - 128 < sidxn <= nshp
    assert wlo <= 32768 and (N + 4) - wlo <= 32768
    assert cap >= nshp and cap % 128 == 0
    return dict(N=N, D=D, E=E, H=H, L=L, ncores=ncores, nsh=nsh, nshp=nshp,
                sidxn=sidxn, wlo=wlo, cap=cap, s=wlo - 2)


FULL_CFG = make_cfg(N=50000, D=128, E=500000, H=3, L=2, ncores=8,
                    wlo=32768, cap=12288)


# ----------------------------------------------------- host preprocessing

def build_schedule(scatter_idx, node_idx, cfg):
    """Bucket edges per (core, hop, pass), degree-sort, build rounds.

    Returns:
      chunks: {(h, p): [ncols, ...]} gather chunk widths (cols, <= cap).
      segs:   {(h, p): [[(off, agg_off, width, is_copy), ...] per chunk]}
              DVE segments addressing each gathered chunk.
      gidx:  [ncores, 128, gcols] int16 gather index data (16-wrapped, 8x
             partition-replicated).
      sidx:  [ncores, 128, H*2*scols] int16 scatter index data.
      seg_cols: per-(hop, pass) (col_off, cols) into gidx free dim.
    """
    N, H, nc_, nsh, nshp = cfg["N"], cfg["H"], cfg["ncores"], cfg["nsh"], cfg["nshp"]
    S, wlo, cap, sidxn = cfg["s"], cfg["wlo"], cfg["cap"], cfg["sidxn"]
    zlo = 0                      # xg row 0 is zeros (lo-window pad index)
    zhi = (N + 3) - wlo          # xg row N+3 is zeros, local to hi window

    # per (hop, pass, core): list of per-round idx arrays (window-local)
    rounds_khp = {}
    orders_khp = {}
    maxpref = {}                 # (h, p) -> list of per-round max prefix
    for h in range(H):
        src_h = np.asarray(scatter_idx[h]).astype(np.int64)
        dst_h = np.asarray(node_idx[h]).astype(np.int64)
        core_of = dst_h // nsh
        for k in range(nc_):
            m = core_of == k
            src_k = src_h[m]
            dst_k = dst_h[m] - k * nsh
            for p in range(2):            # 0 = lo, 1 = hi
                if p == 0:
                    mm = src_k < S
                    ps = src_k[mm] + 2            # window-local row index
                else:
                    mm = src_k >= S
                    ps = src_k[mm] + 2 - wlo
                pd = dst_k[mm]
                deg = np.bincount(pd, minlength=nsh)
                order = np.argsort(-deg, kind="stable")
                pos = np.empty(nsh, np.int64)
                pos[order] = np.arange(nsh)
                key = pos[pd]
                so = np.argsort(key, kind="stable")
                ps_s = ps[so]
                key_s = key[so]
                rank = np.arange(len(key_s)) - np.searchsorted(key_s, key_s)
                rlist = []
                maxdeg = int(deg.max()) if len(deg) else 0
                for j in range(maxdeg):
                    rlist.append(ps_s[rank == j].astype(np.int64))
                rounds_khp[(h, p, k)] = rlist
                orders_khp[(h, p, k)] = order
                mp = maxpref.setdefault((h, p), [])
                for j, r in enumerate(rlist):
                    if j < len(mp):
                        mp[j] = max(mp[j], len(r))
                    else:
                        mp.append(len(r))

    # Column layout per (h, p): round 0 padded to nshp (full-width copy,
    # covers deg-0 tails with zero rows), rounds j>=1 padded to the 128-mult
    # of the max prefix over cores, all concatenated, then split into
    # cap-wide gather chunks.  DVE segments carry (chunk-local offset,
    # position offset within the round, width, is_copy).
    chunks = {}
    segs = {}
    npads = {}
    for h in range(H):
        for p in range(2):
            mp = maxpref.get((h, p), [0])
            npad_l = [nshp]
            for j in range(1, len(mp)):
                if mp[j] > 0:
                    npad_l.append(-(-mp[j] // 128) * 128)
            npads[(h, p)] = npad_l
            total = sum(npad_l)
            cl = []
            o = 0
            while o < total:
                cl.append(min(cap, total - o))
                o += cap
            chunks[(h, p)] = cl
            sl = [[] for _ in cl]
            c = 0
            for j, npad in enumerate(npad_l):
                off = 0
                while off < npad:
                    g = c + off
                    ci = g // cap
                    in_off = g % cap
                    w = min(npad - off, cap - in_off)
                    sl[ci].append((in_off, off, w, j == 0))
                    off += w
                c += npad
            segs[(h, p)] = sl

    # gather idx blobs
    seg_cols = {}
    col = 0
    for h in range(H):
        for p in range(2):
            ncols = sum(npads[(h, p)]) // 16
            seg_cols[(h, p)] = (col, ncols)
            col += ncols
    gcols = col
    gidx = np.zeros((nc_, 128, gcols), np.int16)
    for k in range(nc_):
        for h in range(H):
            for p in range(2):
                zpad = zlo if p == 0 else zhi
                rlist = rounds_khp[(h, p, k)]
                npad_l = npads[(h, p)]
                padded = []
                r0 = rlist[0] if rlist else np.zeros(0, np.int64)
                v = np.full(nshp, zpad, np.int64)
                v[: len(r0)] = r0
                padded.append(v)
                for jj, npad in enumerate(npad_l[1:]):
                    j = jj + 1
                    v = np.full(npad, zpad, np.int64)
                    if j < len(rlist):
                        v[: len(rlist[j])] = rlist[j]
                    padded.append(v)
                blob = np.concatenate(padded)
                c0, ncols = seg_cols[(h, p)]
                assert blob.size == ncols * 16, (blob.size, ncols * 16)
                wrapped = blob.reshape(ncols, 16).T.astype(np.int16)  # [16, ncols]
                gidx[k, :, c0:c0 + ncols] = np.tile(wrapped, (8, 1))

    # scatter idx blobs: per (h, p) a column range of width scols
    scols = cfg["sidxn"] // 16
    sidx = np.zeros((nc_, 128, H * 2 * scols), np.int16)
    for k in range(nc_):
        for h in range(H):
            for p in range(2):
                order = orders_khp[(h, p, k)]
                v = np.full(cfg["sidxn"], -1, np.int64)
                v[:nsh] = order
                wrapped = v.reshape(scols, 16).T.astype(np.int16)
                c0 = (h * 2 + p) * scols
                sidx[k, :, c0:c0 + scols] = np.tile(wrapped, (8, 1))

    return chunks, segs, gidx, sidx, seg_cols


# ------------------------------------------------------- device program

def build_program(cfg, chunks, segs, seg_cols, repeat=1, no_collective=False,
                  loop_repeat=None, no_gather=False, shared_xg2=False):
    import concourse.bacc as bacc
    import concourse.tile as tile
    from concourse import bass, mybir
    from concourse import library_config

    N, D, H, L = cfg["N"], cfg["D"], cfg["H"], cfg["L"]
    nsh, nshp, sidxn, wlo = cfg["nsh"], cfg["nshp"], cfg["sidxn"], cfg["wlo"]
    nc_cores = cfg["ncores"]
    f32 = mybir.dt.float32
    bf16 = mybir.dt.bfloat16
    i16 = mybir.dt.int16
    CH = nshp // 128                      # position chunks (49 full-size)
    GRP = -(-CH // 2) * 128               # accE/accO free width (25*128)
    scols = sidxn // 16
    gcols = max(c0 + nc for (c0, nc) in seg_cols.values())

    nc = bacc.Bacc("TRN2", target_bir_lowering=False, debug=False,
                   num_devices=cfg["ncores"])

    xg_in = nc.dram_tensor("xg", [N + 4, D], bf16, kind="ExternalInput")
    gidx_t = nc.dram_tensor("gidx", [128, gcols], i16, kind="ExternalInput")
    sidx_t = nc.dram_tensor("sidx", [128, H * 2 * scols], i16, kind="ExternalInput")
    xk_t = nc.dram_tensor("xk", [nshp, D], bf16, kind="ExternalInput")
    identf_t = nc.dram_tensor("identf", [128, 128], f32, kind="ExternalInput")
    identb_t = nc.dram_tensor("identb", [128, 128], bf16, kind="ExternalInput")
    w1_t = nc.dram_tensor("w1f", [L * H * D, D], bf16, kind="ExternalInput")
    w2_t = nc.dram_tensor("w2f", [L * H * D, D], bf16, kind="ExternalInput")
    eps1_t = nc.dram_tensor("eps1", [128, 1], f32, kind="ExternalInput")
    out_t = nc.dram_tensor("out", [nshp, D], f32, kind="ExternalOutput")

    xg2 = nc.dram_tensor("xg2", [N + 4, D], bf16,
                         addr_space="Shared" if shared_xg2 else "Local")
    agin = nc.dram_tensor("agin", [nshp, D], bf16)         # internal AG input

    with tile.TileContext(nc) as tc:
        with (
            tc.tile_pool(name="persist", bufs=1) as pp,
            tc.tile_pool(name="xkpool", bufs=1) as xkp,
            tc.tile_pool(name="big", bufs=2) as bigp,
            tc.tile_pool(name="gt", bufs=2) as gtp,
            tc.tile_pool(name="ix", bufs=1) as ixp,
            tc.tile_pool(name="r1p", bufs=2) as r1p,
            tc.tile_pool(name="ps", bufs=2, space="PSUM") as psp,
            tc.tile_pool(name="ps2", bufs=2, space="PSUM") as ps2p,
        ):
            nc.gpsimd.load_library(library_config.mlp)
            _regs = {}

            def nreg(v):
                if v not in _regs:
                    _regs[v] = nc.gpsimd.to_reg(v)
                return _regs[v]

            identf = pp.tile([128, 128], f32, tag="identf")
            nc.sync.dma_start(identf[:], identf_t[:, :])
            identb = pp.tile([128, 128], bf16, tag="identb")
            nc.sync.dma_start(identb[:], identb_t[:, :])
            eps1 = pp.tile([128, 1], f32, tag="eps1")
            nc.sync.dma_start(eps1[:], eps1_t[:, :])
            sidx_sb = pp.tile([128, H * 2 * scols], i16, tag="sidx")
            nc.sync.dma_start(sidx_sb[:], sidx_t[:, :])
            wtiles = {}
            for l in range(L):
                for h in range(H):
                    wt1 = pp.tile([128, D], bf16, tag=f"w1_{l}_{h}")
                    wt2 = pp.tile([128, D], bf16, tag=f"w2_{l}_{h}")
                    lh = l * H + h
                    nc.sync.dma_start(wt1[:], w1_t[lh * D:(lh + 1) * D, :])
                    nc.sync.dma_start(wt2[:], w2_t[lh * D:(lh + 1) * D, :])
                    wtiles[(l, h)] = (wt1, wt2)

            # zero the pad rows of xg2 (rows 0,1 and N+2,N+3)
            ztile = pp.tile([2, D], bf16, tag="zz")
            nc.vector.memset(ztile[:], 0.0)
            nc.sync.dma_start(xg2[0:2, :], ztile[:])
            nc.sync.dma_start(xg2[N + 2:N + 4, :], ztile[:])

            accE = pp.tile([128, GRP], bf16, tag="accE")
            accO = pp.tile([128, GRP], bf16, tag="accO")
            outaccT = pp.tile([128, nshp], f32, tag="outaccT")
            outf = pp.tile([128, nshp], f32, tag="outf")

            import contextlib
            rep_ctx = (tc.For_i(0, loop_repeat, 1) if loop_repeat
                       else contextlib.nullcontext())
            with rep_ctx:
             for rep in range(repeat):
              xk_sb = xkp.tile([128, nshp], bf16, tag="xk")
              nc.sync.dma_start(
                  xk_sb[:].rearrange("p (c e) -> p c e", e=D),
                  xk_t.ap().rearrange("(c p) e -> p c e", p=128),
              )

              for l in range(L):
                xsrc = xg_in if l == 0 else xg2
                win = {0: xsrc[0:wlo, :], 1: xsrc[wlo:N + 4, :]}
                xkc3 = xk_sb[:].rearrange("p (c e) -> p c e", e=D)

                # outaccT = (1+eps) * x_k^T
                if l == 0:
                    tp = None
                    for c in range(CH):
                        q = c % 4
                        if q == 0:
                            tp = psp.tile([128, 512], bf16, tag="tp")
                        nc.tensor.transpose(
                            tp[:, q * 128:(q + 1) * 128],
                            xk_sb[:, c * 128:(c + 1) * 128], identb[:])
                        if q == 3 or c == CH - 1:
                            w = (q + 1) * 128
                            nc.scalar.activation(
                                outaccT[:, (c - q) * 128:(c - q) * 128 + w],
                                tp[:, :w], mybir.ActivationFunctionType.Copy,
                                scale=eps1[:, 0:1])
                else:
                    # outaccT already holds x^T (pre-back-transpose value):
                    # scale in place.
                    o = 0
                    while o < nshp:
                        w = min(512, nshp - o)
                        nc.scalar.activation(
                            outaccT[:, o:o + w], outaccT[:, o:o + w],
                            mybir.ActivationFunctionType.Copy,
                            scale=eps1[:, 0:1])
                        o += w

                for h in range(H):
                    # canonical accumulators = x_k (even / odd chunks)
                    nE = (CH + 1) // 2
                    nO = CH // 2
                    nc.vector.tensor_copy(
                        accE[:].rearrange("p (c e) -> p c e", e=D)[:, 0:nE, :],
                        xkc3[:, 0:CH:2, :])
                    nc.vector.tensor_copy(
                        accO[:].rearrange("p (c e) -> p c e", e=D)[:, 0:nO, :],
                        xkc3[:, 1:CH:2, :])

                    for p in range(2):
                        c0, ncols = seg_cols[(h, p)]
                        iseg = ixp.tile([128, ncols], i16, tag="iseg")
                        nc.sync.dma_start(iseg[:], gidx_t[:, c0:c0 + ncols])
                        agg = bigp.tile([128, nshp], bf16, tag="big")
                        icol = 0
                        for ci, ncol in enumerate(chunks[(h, p)]):
                            gt = gtp.tile([128, cfg["cap"]], bf16, tag="gt")
                            if not no_gather:
                                nc.gpsimd.dma_gather(
                                    gt[:, 0:ncol].rearrange(
                                        "p (g e) -> p g e", e=D),
                                    win[p],
                                    iseg[:, icol:icol + ncol // 16],
                                    ncol, nreg(ncol), D, single_packet=False)
                            for (off, agg_off, w, is_copy) in segs[(h, p)][ci]:
                                if is_copy:
                                    nc.vector.tensor_copy(
                                        agg[:, agg_off:agg_off + w],
                                        gt[:, off:off + w])
                                else:
                                    nc.vector.tensor_tensor(
                                        agg[:, agg_off:agg_off + w],
                                        agg[:, agg_off:agg_off + w],
                                        gt[:, off:off + w],
                                        mybir.AluOpType.add)
                            icol += ncol // 16
                        sc0 = (h * 2 + p) * scols
                        nc.gpsimd.dma_scatter_add(
                            accE[:].rearrange("p (c e) -> p c e", e=D),
                            agg[:].rearrange("p (c e) -> p c e", e=D),
                            sidx_sb[:, sc0:sc0 + scols],
                            sidxn, nreg(nsh), D,
                            single_packet=False,
                            sbuf_tokens_per_rank=128,
                            parity_reg=nreg(0),
                            out_ap_other=accO[:].rearrange(
                                "p (c e) -> p c e", e=D))

                    # transpose x+agg into feature-major xpaT
                    xpaT = bigp.tile([128, nshp], bf16, tag="big")
                    tp = None
                    for c in range(CH):
                        q = c % 4
                        if q == 0:
                            tp = psp.tile([128, 512], bf16, tag="tp")
                        buf = accE if c % 2 == 0 else accO
                        g = c // 2
                        nc.tensor.transpose(
                            tp[:, q * 128:(q + 1) * 128],
                            buf[:, g * 128:(g + 1) * 128], identb[:])
                        if q == 3 or c == CH - 1:
                            w = (q + 1) * 128
                            nc.scalar.activation(
                                xpaT[:, (c - q) * 128:(c - q) * 128 + w],
                                tp[:, :w], mybir.ActivationFunctionType.Copy)

                    # MLP: out += relu(xpa @ w1) @ w2   (feature-major)
                    wt1, wt2 = wtiles[(l, h)]
                    o = 0
                    while o < nshp:
                        w = min(512, nshp - o)
                        ps1 = psp.tile([128, 512], f32, tag="mm1")
                        nc.tensor.matmul(ps1[:, :w], wt1[:], xpaT[:, o:o + w],
                                         start=True, stop=True)
                        r1 = r1p.tile([128, 512], bf16, tag="r1")
                        nc.scalar.activation(
                            r1[:, :w], ps1[:, :w],
                            mybir.ActivationFunctionType.Relu)
                        ps2 = ps2p.tile([128, 512], f32, tag="mm2")
                        nc.tensor.matmul(ps2[:, :w], wt2[:], r1[:, :w],
                                         start=True, stop=True)
                        nc.vector.tensor_tensor(
                            outaccT[:, o:o + w], outaccT[:, o:o + w],
                            ps2[:, :w], mybir.AluOpType.add)
                        o += w

                # back-transpose outaccT (f32) -> node-major new x
                if l == 0:
                    xk_new = xkp.tile([128, nshp], bf16, tag="xk")
                    tp = None
                    for c in range(CH):
                        q = c % 4
                        if q == 0:
                            tp = psp.tile([128, 512], f32, tag="tpf")
                        nc.tensor.transpose(
                            tp[:, q * 128:(q + 1) * 128],
                            outaccT[:, c * 128:(c + 1) * 128], identf[:])
                        if q == 3 or c == CH - 1:
                            w = (q + 1) * 128
                            nc.scalar.activation(
                                xk_new[:, (c - q) * 128:(c - q) * 128 + w],
                                tp[:, :w], mybir.ActivationFunctionType.Copy)
                    xk_sb = xk_new
                    nc.sync.dma_start(
                        agin.ap().rearrange("(c p) e -> p c e", p=128),
                        xk_sb[:].rearrange("p (c e) -> p c e", e=D))
                    if no_collective:
                        nc.sync.dma_start(xg2[2:2 + nsh, :], agin[0:nsh, :])
                    else:
                        nc.gpsimd.collective_compute(
                            "AllGather", mybir.AluOpType.bypass,
                            replica_groups=[list(range(nc_cores))],
                            ins=[agin[0:nsh, :]],
                            outs=[xg2[2:2 + N, :]])
                else:
                    tp = None
                    for c in range(CH):
                        q = c % 4
                        if q == 0:
                            tp = psp.tile([128, 512], f32, tag="tpf")
                        nc.tensor.transpose(
                            tp[:, q * 128:(q + 1) * 128],
                            outaccT[:, c * 128:(c + 1) * 128], identf[:])
                        if q == 3 or c == CH - 1:
                            w = (q + 1) * 128
                            nc.scalar.activation(
                                outf[:, (c - q) * 128:(c - q) * 128 + w],
                                tp[:, :w], mybir.ActivationFunctionType.Copy)
                    nc.sync.dma_start(
                        out_t.ap().rearrange("(c p) e -> p c e", p=128),
                        outf[:].rearrange("p (c e) -> p c e", e=D))

    nc.compile()
    return nc


# ------------------------------------------------------------- entry

def _prep_inputs(x, w1, w2, eps, scatter_idx, node_idx, cfg):
    import ml_dtypes
    bf16 = ml_dtypes.bfloat16
    N, D, H, L, nc_ = cfg["N"], cfg["D"], cfg["H"], cfg["L"], cfg["ncores"]
    nsh, nshp = cfg["nsh"], cfg["nshp"]
    x = np.asarray(x, np.float32)
    chunks, segs, gidx, sidx, seg_cols = build_schedule(
        scatter_idx, node_idx, cfg)
    xg = np.zeros((N + 4, D), bf16)
    xg[2:2 + N] = x.astype(bf16)
    w1f = np.asarray(w1, np.float32).reshape(L * H * D, D).astype(bf16)
    w2f = np.asarray(w2, np.float32).reshape(L * H * D, D).astype(bf16)
    eps1 = np.full((128, 1), 1.0 + float(np.asarray(eps).reshape(-1)[0]),
                   np.float32)
    in_maps = []
    for k in range(nc_):
        xk = np.zeros((nshp, D), bf16)
        xk[:nsh] = x[k * nsh:(k + 1) * nsh].astype(bf16)
        in_maps.append({
            "xg": xg, "gidx": gidx[k], "sidx": sidx[k], "xk": xk,
            "w1f": w1f, "w2f": w2f, "eps1": eps1,
            "identf": np.eye(128, dtype=np.float32),
            "identb": np.eye(128).astype(bf16),
        })
    return (chunks, segs, seg_cols), in_maps


def kernel_with_results(x, w1, w2, eps, scatter_idx, node_idx, cfg=None,
                        **run_kwargs):
    cfg = cfg or FULL_CFG
    (chunks, segs, seg_cols), in_maps = _prep_inputs(
        x, w1, w2, eps, scatter_idx, node_idx, cfg)
    nc = build_program(cfg, chunks, segs, seg_cols)

    from concourse.bass_utils import run_bass_kernel_spmd
    res = run_bass_kernel_spmd(nc, in_maps,
                               core_ids=list(range(cfg["ncores"])),
                               **run_kwargs)
    outs = [res.results[k]["out"][:cfg["nsh"]] for k in range(cfg["ncores"])]
    return np.concatenate(outs, axis=0).astype(np.float32), res


def kernel(x, w1, w2, eps, scatter_idx, node_idx):
    out, _ = kernel_with_results(x, w1, w2, eps, scatter_idx, node_idx)
    return out


# revision 15
# speedup vs baseline: 1.0373x; 1.0373x over previous
"""Trainium2 Bass kernel for nn_LocalWLGNN (gnn_message_passing).

Reference computation (per layer l, x: [N, D]):
    out = (1+eps) * x
    for hop in range(H):
        agg = segment_sum(x[scatter_idx[hop]], node_idx[hop], N)
        out += relu((x + agg) @ w1[l,hop]) @ w2[l,hop]
    x = out

Sharding: 8 cores, core k owns destination nodes [k*N/8, (k+1)*N/8).
Each (core, hop) edge list is split by source-node window (lo/hi, so row
indices fit the int16 dma_gather contract), and within each pass the
destinations are sorted by in-degree so that "round j = j-th in-edge of
every destination" covers a contiguous position prefix.  On device:
  - dma_gather (SWDGE) fetches rows from DRAM in large merged chunks
    (rounds concatenated, split only at the chunk capacity),
  - DVE accumulates round segments into a per-pass aggregate (pass order),
  - dma_scatter_add (SBUF parity-split CCE mode) permutes+adds both pass
    aggregates back into canonical order on top of x_k,
  - PE transposes to feature-major, runs the 2-layer MLP,
  - hop outputs accumulate into a feature-major f32 accumulator,
  - an 8-core bf16 AllGather (Shared-output fast path) republishes the
    new node features between layers.
The x-path (features, gather, aggregate, MLP operands) runs in bfloat16;
hop-sum accumulation and the final output stay float32.
"""

import numpy as np


# ---------------------------------------------------------------- config

def make_cfg(N, D, E, H, L, ncores, wlo, cap):
    nsh = N // ncores
    nshp = -(-nsh // 128) * 128          # padded positions per core
    sidxn = -(-nsh // 16) * 16           # scatter num_idxs (16-mult)
    assert nshp - 128 < sidxn <= nshp
    assert wlo <= 32768 and (N + 4) - wlo <= 32768
    assert cap >= nshp and cap % 128 == 0
    return dict(N=N, D=D, E=E, H=H, L=L, ncores=ncores, nsh=nsh, nshp=nshp,
                sidxn=sidxn, wlo=wlo, cap=cap, s=wlo - 2)


FULL_CFG = make_cfg(N=50000, D=128, E=500000, H=3, L=2, ncores=8,
                    wlo=32768, cap=12288)


# ----------------------------------------------------- host preprocessing

def build_schedule(scatter_idx, node_idx, cfg):
    """Bucket edges per (core, hop, pass), degree-sort, build rounds.

    Returns:
      chunks: {(h, p): [ncols, ...]} gather chunk widths (cols, <= cap).
      segs:   {(h, p): [[(off, agg_off, width, is_copy), ...] per chunk]}
              DVE segments addressing each gathered chunk.
      gidx:  [ncores, 128, gcols] int16 gather index data (16-wrapped, 8x
             partition-replicated).
      sidx:  [ncores, 128, H*2*scols] int16 scatter index data.
      seg_cols: per-(hop, pass) (col_off, cols) into gidx free dim.
    """
    N, H, nc_, nsh, nshp = cfg["N"], cfg["H"], cfg["ncores"], cfg["nsh"], cfg["nshp"]
    S, wlo, cap, sidxn = cfg["s"], cfg["wlo"], cfg["cap"], cfg["sidxn"]
    zlo = 0                      # xg row 0 is zeros (lo-window pad index)
    zhi = (N + 3) - wlo          # xg row N+3 is zeros, local to hi window

    # per (hop, pass, core): list of per-round idx arrays (window-local)
    rounds_khp = {}
    orders_khp = {}
    maxpref = {}                 # (h, p) -> list of per-round max prefix
    for h in range(H):
        src_h = np.asarray(scatter_idx[h]).astype(np.int64)
        dst_h = np.asarray(node_idx[h]).astype(np.int64)
        core_of = dst_h // nsh
        for k in range(nc_):
            m = core_of == k
            src_k = src_h[m]
            dst_k = dst_h[m] - k * nsh
            for p in range(2):            # 0 = lo, 1 = hi
                if p == 0:
                    mm = src_k < S
                    ps = src_k[mm] + 2            # window-local row index
                else:
                    mm = src_k >= S
                    ps = src_k[mm] + 2 - wlo
                pd = dst_k[mm]
                deg = np.bincount(pd, minlength=nsh)
                order = np.argsort(-deg, kind="stable")
                pos = np.empty(nsh, np.int64)
                pos[order] = np.arange(nsh)
                key = pos[pd]
                so = np.argsort(key, kind="stable")
                ps_s = ps[so]
                key_s = key[so]
                rank = np.arange(len(key_s)) - np.searchsorted(key_s, key_s)
                rlist = []
                maxdeg = int(deg.max()) if len(deg) else 0
                for j in range(maxdeg):
                    rlist.append(ps_s[rank == j].astype(np.int64))
                rounds_khp[(h, p, k)] = rlist
                orders_khp[(h, p, k)] = order
                mp = maxpref.setdefault((h, p), [])
                for j, r in enumerate(rlist):
                    if j < len(mp):
                        mp[j] = max(mp[j], len(r))
                    else:
                        mp.append(len(r))

    # Column layout per (h, p): round 0 padded to nshp (full-width copy,
    # covers deg-0 tails with zero rows), rounds j>=1 padded to the 128-mult
    # of the max prefix over cores, all concatenated, then split into
    # cap-wide gather chunks.  DVE segments carry (chunk-local offset,
    # position offset within the round, width, is_copy).
    chunks = {}
    segs = {}
    npads = {}
    for h in range(H):
        for p in range(2):
            mp = maxpref.get((h, p), [0])
            npad_l = [nshp]
            for j in range(1, len(mp)):
                if mp[j] > 0:
                    npad_l.append(-(-mp[j] // 128) * 128)
            npads[(h, p)] = npad_l
            total = sum(npad_l)
            cl = []
            o = 0
            while o < total:
                cl.append(min(cap, total - o))
                o += cap
            chunks[(h, p)] = cl
            sl = [[] for _ in cl]
            c = 0
            for j, npad in enumerate(npad_l):
                off = 0
                while off < npad:
                    g = c + off
                    ci = g // cap
                    in_off = g % cap
                    w = min(npad - off, cap - in_off)
                    sl[ci].append((in_off, off, w, j == 0))
                    off += w
                c += npad
            segs[(h, p)] = sl

    # gather idx blobs
    seg_cols = {}
    col = 0
    for h in range(H):
        for p in range(2):
            ncols = sum(npads[(h, p)]) // 16
            seg_cols[(h, p)] = (col, ncols)
            col += ncols
    gcols = col
    gidx = np.zeros((nc_, 128, gcols), np.int16)
    for k in range(nc_):
        for h in range(H):
            for p in range(2):
                zpad = zlo if p == 0 else zhi
                rlist = rounds_khp[(h, p, k)]
                npad_l = npads[(h, p)]
                padded = []
                r0 = rlist[0] if rlist else np.zeros(0, np.int64)
                v = np.full(nshp, zpad, np.int64)
                v[: len(r0)] = r0
                padded.append(v)
                for jj, npad in enumerate(npad_l[1:]):
                    j = jj + 1
                    v = np.full(npad, zpad, np.int64)
                    if j < len(rlist):
                        v[: len(rlist[j])] = rlist[j]
                    padded.append(v)
                blob = np.concatenate(padded)
                c0, ncols = seg_cols[(h, p)]
                assert blob.size == ncols * 16, (blob.size, ncols * 16)
                wrapped = blob.reshape(ncols, 16).T.astype(np.int16)  # [16, ncols]
                gidx[k, :, c0:c0 + ncols] = np.tile(wrapped, (8, 1))

    # scatter idx blobs: per (h, p) a column range of width scols
    scols = cfg["sidxn"] // 16
    sidx = np.zeros((nc_, 128, H * 2 * scols), np.int16)
    for k in range(nc_):
        for h in range(H):
            for p in range(2):
                order = orders_khp[(h, p, k)]
                v = np.full(cfg["sidxn"], -1, np.int64)
                v[:nsh] = order
                wrapped = v.reshape(scols, 16).T.astype(np.int16)
                c0 = (h * 2 + p) * scols
                sidx[k, :, c0:c0 + scols] = np.tile(wrapped, (8, 1))

    return chunks, segs, gidx, sidx, seg_cols


# ------------------------------------------------------- device program

def build_program(cfg, chunks, segs, seg_cols, repeat=1, no_collective=False,
                  loop_repeat=None, no_gather=False, shared_xg2=False,
                  tiny_out=False, nqueues=4, sp=False):
    import concourse.bacc as bacc
    import concourse.tile as tile
    from concourse import bass, mybir
    from concourse import library_config

    N, D, H, L = cfg["N"], cfg["D"], cfg["H"], cfg["L"]
    nsh, nshp, sidxn, wlo = cfg["nsh"], cfg["nshp"], cfg["sidxn"], cfg["wlo"]
    nc_cores = cfg["ncores"]
    f32 = mybir.dt.float32
    bf16 = mybir.dt.bfloat16
    i16 = mybir.dt.int16
    CH = nshp // 128                      # position chunks (49 full-size)
    GRP = -(-CH // 2) * 128               # accE/accO free width (25*128)
    scols = sidxn // 16
    gcols = max(c0 + nc for (c0, nc) in seg_cols.values())

    nc = bacc.Bacc("TRN2", target_bir_lowering=False, debug=False,
                   num_devices=cfg["ncores"], num_swdge_queues=nqueues)
    qctr = [0]

    def nextq():
        q = qctr[0] % nqueues
        qctr[0] += 1
        return q

    xg_in = nc.dram_tensor("xg", [N + 4, D], bf16, kind="ExternalInput")
    gidx_t = nc.dram_tensor("gidx", [128, gcols], i16, kind="ExternalInput")
    sidx_t = nc.dram_tensor("sidx", [128, H * 2 * scols], i16, kind="ExternalInput")
    xk_t = nc.dram_tensor("xk", [nshp, D], bf16, kind="ExternalInput")
    identf_t = nc.dram_tensor("identf", [128, 128], f32, kind="ExternalInput")
    identb_t = nc.dram_tensor("identb", [128, 128], bf16, kind="ExternalInput")
    w1_t = nc.dram_tensor("w1f", [L * H * D, D], bf16, kind="ExternalInput")
    w2_t = nc.dram_tensor("w2f", [L * H * D, D], bf16, kind="ExternalInput")
    eps1_t = nc.dram_tensor("eps1", [128, 1], f32, kind="ExternalInput")
    out_t = nc.dram_tensor("out", [128 if tiny_out else nshp, D], f32,
                           kind="ExternalOutput")

    xg2 = nc.dram_tensor("xg2", [N + 4, D], bf16,
                         addr_space="Shared" if shared_xg2 else "Local")
    agin = nc.dram_tensor("agin", [nshp, D], bf16)         # internal AG input

    with tile.TileContext(nc) as tc:
        with (
            tc.tile_pool(name="persist", bufs=1) as pp,
            tc.tile_pool(name="xkpool", bufs=1) as xkp,
            tc.tile_pool(name="big", bufs=2) as bigp,
            tc.tile_pool(name="gt", bufs=2) as gtp,
            tc.tile_pool(name="ix", bufs=1) as ixp,
            tc.tile_pool(name="r1p", bufs=2) as r1p,
            tc.tile_pool(name="ps", bufs=2, space="PSUM") as psp,
            tc.tile_pool(name="ps2", bufs=2, space="PSUM") as ps2p,
        ):
            nc.gpsimd.load_library(library_config.mlp)
            _regs = {}

            def nreg(v):
                if v not in _regs:
                    _regs[v] = nc.gpsimd.to_reg(v)
                return _regs[v]

            identf = pp.tile([128, 128], f32, tag="identf")
            nc.sync.dma_start(identf[:], identf_t[:, :])
            identb = pp.tile([128, 128], bf16, tag="identb")
            nc.sync.dma_start(identb[:], identb_t[:, :])
            eps1 = pp.tile([128, 1], f32, tag="eps1")
            nc.sync.dma_start(eps1[:], eps1_t[:, :])
            sidx_sb = pp.tile([128, H * 2 * scols], i16, tag="sidx")
            nc.sync.dma_start(sidx_sb[:], sidx_t[:, :])
            wtiles = {}
            for l in range(L):
                for h in range(H):
                    wt1 = pp.tile([128, D], bf16, tag=f"w1_{l}_{h}")
                    wt2 = pp.tile([128, D], bf16, tag=f"w2_{l}_{h}")
                    lh = l * H + h
                    nc.sync.dma_start(wt1[:], w1_t[lh * D:(lh + 1) * D, :])
                    nc.sync.dma_start(wt2[:], w2_t[lh * D:(lh + 1) * D, :])
                    wtiles[(l, h)] = (wt1, wt2)

            # zero the pad rows of xg2 (rows 0,1 and N+2,N+3)
            ztile = pp.tile([2, D], bf16, tag="zz")
            nc.vector.memset(ztile[:], 0.0)
            nc.sync.dma_start(xg2[0:2, :], ztile[:])
            nc.sync.dma_start(xg2[N + 2:N + 4, :], ztile[:])

            accE = pp.tile([128, GRP], bf16, tag="accE")
            accO = pp.tile([128, GRP], bf16, tag="accO")
            outaccT = pp.tile([128, nshp], f32, tag="outaccT")
            outf = pp.tile([128, nshp], f32, tag="outf")

            import contextlib
            rep_ctx = (tc.For_i(0, loop_repeat, 1) if loop_repeat
                       else contextlib.nullcontext())
            with rep_ctx:
             for rep in range(repeat):
              xk_sb = xkp.tile([128, nshp], bf16, tag="xk")
              nc.sync.dma_start(
                  xk_sb[:].rearrange("p (c e) -> p c e", e=D),
                  xk_t.ap().rearrange("(c p) e -> p c e", p=128),
              )

              for l in range(L):
                xsrc = xg_in if l == 0 else xg2
                win = {0: xsrc[0:wlo, :], 1: xsrc[wlo:N + 4, :]}
                xkc3 = xk_sb[:].rearrange("p (c e) -> p c e", e=D)

                # outaccT = (1+eps) * x_k^T
                if l == 0:
                    tp = None
                    for c in range(CH):
                        q = c % 4
                        if q == 0:
                            tp = psp.tile([128, 512], bf16, tag="tp")
                        nc.tensor.transpose(
                            tp[:, q * 128:(q + 1) * 128],
                            xk_sb[:, c * 128:(c + 1) * 128], identb[:])
                        if q == 3 or c == CH - 1:
                            w = (q + 1) * 128
                            nc.scalar.activation(
                                outaccT[:, (c - q) * 128:(c - q) * 128 + w],
                                tp[:, :w], mybir.ActivationFunctionType.Copy,
                                scale=eps1[:, 0:1])
                else:
                    # outaccT already holds x^T (pre-back-transpose value):
                    # scale in place.
                    o = 0
                    while o < nshp:
                        w = min(512, nshp - o)
                        nc.scalar.activation(
                            outaccT[:, o:o + w], outaccT[:, o:o + w],
                            mybir.ActivationFunctionType.Copy,
                            scale=eps1[:, 0:1])
                        o += w

                for h in range(H):
                    # canonical accumulators = x_k (even / odd chunks)
                    nE = (CH + 1) // 2
                    nO = CH // 2
                    nc.vector.tensor_copy(
                        accE[:].rearrange("p (c e) -> p c e", e=D)[:, 0:nE, :],
                        xkc3[:, 0:CH:2, :])
                    nc.vector.tensor_copy(
                        accO[:].rearrange("p (c e) -> p c e", e=D)[:, 0:nO, :],
                        xkc3[:, 1:CH:2, :])

                    for p in range(2):
                        c0, ncols = seg_cols[(h, p)]
                        iseg = ixp.tile([128, ncols], i16, tag="iseg")
                        nc.sync.dma_start(iseg[:], gidx_t[:, c0:c0 + ncols])
                        agg = bigp.tile([128, nshp], bf16, tag="big")
                        icol = 0
                        for ci, ncol in enumerate(chunks[(h, p)]):
                            gt = gtp.tile([128, cfg["cap"]], bf16, tag="gt")
                            if no_gather:
                                nc.vector.memset(gt[:, 0:1], 0.0)
                            else:
                                nc.gpsimd.dma_gather(
                                    gt[:, 0:ncol].rearrange(
                                        "p (g e) -> p g e", e=D),
                                    win[p],
                                    iseg[:, icol:icol + ncol // 16],
                                    ncol, nreg(ncol), D, single_packet=sp,
                                    queue_num=nextq())
                            for (off, agg_off, w, is_copy) in segs[(h, p)][ci]:
                                if is_copy:
                                    nc.vector.tensor_copy(
                                        agg[:, agg_off:agg_off + w],
                                        gt[:, off:off + w])
                                else:
                                    nc.vector.tensor_tensor(
                                        agg[:, agg_off:agg_off + w],
                                        agg[:, agg_off:agg_off + w],
                                        gt[:, off:off + w],
                                        mybir.AluOpType.add)
                            icol += ncol // 16
                        sc0 = (h * 2 + p) * scols
                        nc.gpsimd.dma_scatter_add(
                            accE[:].rearrange("p (c e) -> p c e", e=D),
                            agg[:].rearrange("p (c e) -> p c e", e=D),
                            sidx_sb[:, sc0:sc0 + scols],
                            sidxn, nreg(nsh), D,
                            single_packet=False,
                            queue_num=nextq(),
                            sbuf_tokens_per_rank=128,
                            parity_reg=nreg(0),
                            out_ap_other=accO[:].rearrange(
                                "p (c e) -> p c e", e=D))

                    # transpose x+agg into feature-major xpaT
                    xpaT = bigp.tile([128, nshp], bf16, tag="big")
                    tp = None
                    for c in range(CH):
                        q = c % 4
                        if q == 0:
                            tp = psp.tile([128, 512], bf16, tag="tp")
                        buf = accE if c % 2 == 0 else accO
                        g = c // 2
                        nc.tensor.transpose(
                            tp[:, q * 128:(q + 1) * 128],
                            buf[:, g * 128:(g + 1) * 128], identb[:])
                        if q == 3 or c == CH - 1:
                            w = (q + 1) * 128
                            nc.scalar.activation(
                                xpaT[:, (c - q) * 128:(c - q) * 128 + w],
                                tp[:, :w], mybir.ActivationFunctionType.Copy)

                    # MLP: out += relu(xpa @ w1) @ w2   (feature-major)
                    wt1, wt2 = wtiles[(l, h)]
                    o = 0
                    while o < nshp:
                        w = min(512, nshp - o)
                        ps1 = psp.tile([128, 512], f32, tag="mm1")
                        nc.tensor.matmul(ps1[:, :w], wt1[:], xpaT[:, o:o + w],
                                         start=True, stop=True)
                        r1 = r1p.tile([128, 512], bf16, tag="r1")
                        nc.scalar.activation(
                            r1[:, :w], ps1[:, :w],
                            mybir.ActivationFunctionType.Relu)
                        ps2 = ps2p.tile([128, 512], f32, tag="mm2")
                        nc.tensor.matmul(ps2[:, :w], wt2[:], r1[:, :w],
                                         start=True, stop=True)
                        nc.vector.tensor_tensor(
                            outaccT[:, o:o + w], outaccT[:, o:o + w],
                            ps2[:, :w], mybir.AluOpType.add)
                        o += w

                # back-transpose outaccT (f32) -> node-major new x
                if l == 0:
                    xk_new = xkp.tile([128, nshp], bf16, tag="xk")
                    tp = None
                    for c in range(CH):
                        q = c % 4
                        if q == 0:
                            tp = psp.tile([128, 512], f32, tag="tpf")
                        nc.tensor.transpose(
                            tp[:, q * 128:(q + 1) * 128],
                            outaccT[:, c * 128:(c + 1) * 128], identf[:])
                        if q == 3 or c == CH - 1:
                            w = (q + 1) * 128
                            nc.scalar.activation(
                                xk_new[:, (c - q) * 128:(c - q) * 128 + w],
                                tp[:, :w], mybir.ActivationFunctionType.Copy)
                    xk_sb = xk_new
                    nc.sync.dma_start(
                        agin.ap().rearrange("(c p) e -> p c e", p=128),
                        xk_sb[:].rearrange("p (c e) -> p c e", e=D))
                    if no_collective:
                        nc.sync.dma_start(xg2[2:2 + nsh, :], agin[0:nsh, :])
                    else:
                        nc.gpsimd.collective_compute(
                            "AllGather", mybir.AluOpType.bypass,
                            replica_groups=[list(range(nc_cores))],
                            ins=[agin[0:nsh, :]],
                            outs=[xg2[2:2 + N, :]])
                else:
                    tp = None
                    for c in range(CH):
                        q = c % 4
                        if q == 0:
                            tp = psp.tile([128, 512], f32, tag="tpf")
                        nc.tensor.transpose(
                            tp[:, q * 128:(q + 1) * 128],
                            outaccT[:, c * 128:(c + 1) * 128], identf[:])
                        if q == 3 or c == CH - 1:
                            w = (q + 1) * 128
                            nc.scalar.activation(
                                outf[:, (c - q) * 128:(c - q) * 128 + w],
                                tp[:, :w], mybir.ActivationFunctionType.Copy)
                    if tiny_out:
                        nc.sync.dma_start(
                            out_t.ap().rearrange("(c p) e -> p c e", p=128),
                            outf[:, 0:D].rearrange("p (c e) -> p c e", e=D))
                    else:
                        nc.sync.dma_start(
                            out_t.ap().rearrange("(c p) e -> p c e", p=128),
                            outf[:].rearrange("p (c e) -> p c e", e=D))

    nc.compile()
    return nc


# ------------------------------------------------------------- entry

def _prep_inputs(x, w1, w2, eps, scatter_idx, node_idx, cfg):
    import os

    import ml_dtypes
    bf16 = ml_dtypes.bfloat16
    N, D, H, L, nc_ = cfg["N"], cfg["D"], cfg["H"], cfg["L"], cfg["ncores"]
    nsh, nshp = cfg["nsh"], cfg["nshp"]
    x = np.asarray(x, np.float32)
    chunks, segs, gidx, sidx, seg_cols = build_schedule(
        scatter_idx, node_idx, cfg)
    if os.environ.get("ZERO_GIDX"):
        gidx[:] = 0          # same descriptor stream, zero randomness
    if os.environ.get("SEQ_GIDX"):
        # window-local sequential indices: same desc count, fully coalesced
        cols = gidx.shape[2]
        blob = np.arange(cols * 16, dtype=np.int64) % 16384
        w16 = blob.reshape(cols, 16).T.astype(np.int16)
        gidx = np.ascontiguousarray(
            np.broadcast_to(np.tile(w16, (8, 1)), gidx.shape))
    xg = np.zeros((N + 4, D), bf16)
    xg[2:2 + N] = x.astype(bf16)
    w1f = np.asarray(w1, np.float32).reshape(L * H * D, D).astype(bf16)
    w2f = np.asarray(w2, np.float32).reshape(L * H * D, D).astype(bf16)
    eps1 = np.full((128, 1), 1.0 + float(np.asarray(eps).reshape(-1)[0]),
                   np.float32)
    in_maps = []
    for k in range(nc_):
        xk = np.zeros((nshp, D), bf16)
        xk[:nsh] = x[k * nsh:(k + 1) * nsh].astype(bf16)
        in_maps.append({
            "xg": xg, "gidx": gidx[k], "sidx": sidx[k], "xk": xk,
            "w1f": w1f, "w2f": w2f, "eps1": eps1,
            "identf": np.eye(128, dtype=np.float32),
            "identb": np.eye(128).astype(bf16),
        })
    return (chunks, segs, seg_cols), in_maps


def kernel_with_results(x, w1, w2, eps, scatter_idx, node_idx, cfg=None,
                        **run_kwargs):
    cfg = cfg or FULL_CFG
    (chunks, segs, seg_cols), in_maps = _prep_inputs(
        x, w1, w2, eps, scatter_idx, node_idx, cfg)
    nc = build_program(cfg, chunks, segs, seg_cols)

    from concourse.bass_utils import run_bass_kernel_spmd
    res = run_bass_kernel_spmd(nc, in_maps,
                               core_ids=list(range(cfg["ncores"])),
                               **run_kwargs)
    outs = [res.results[k]["out"][:cfg["nsh"]] for k in range(cfg["ncores"])]
    return np.concatenate(outs, axis=0).astype(np.float32), res


def kernel(x, w1, w2, eps, scatter_idx, node_idx):
    out, _ = kernel_with_results(x, w1, w2, eps, scatter_idx, node_idx)
    return out


# revision 17
# speedup vs baseline: 2.0554x; 1.9815x over previous
"""Trainium2 Bass kernel for nn_LocalWLGNN (gnn_message_passing).

Reference computation (per layer l, x: [N, D]):
    out = (1+eps) * x
    for hop in range(H):
        agg = segment_sum(x[scatter_idx[hop]], node_idx[hop], N)
        out += relu((x + agg) @ w1[l,hop]) @ w2[l,hop]
    x = out

Sharding: 8 cores, core k owns destination nodes [k*N/8, (k+1)*N/8).
Each (core, hop) edge list is split by source-node window (lo/hi, so row
indices fit the int16 dma_gather contract), and within each pass the
destinations are sorted by in-degree so that "round j = j-th in-edge of
every destination" covers a contiguous position prefix.  On device:
  - dma_gather (SWDGE) fetches rows from DRAM in large merged chunks
    (rounds concatenated, split only at the chunk capacity),
  - DVE accumulates round segments into a per-pass aggregate (pass order),
  - dma_scatter_add (SBUF parity-split CCE mode) permutes+adds both pass
    aggregates back into canonical order on top of x_k,
  - PE transposes to feature-major, runs the 2-layer MLP,
  - hop outputs accumulate into a feature-major f32 accumulator,
  - an 8-core bf16 AllGather (Shared-output fast path) republishes the
    new node features between layers.
The x-path (features, gather, aggregate, MLP operands) runs in bfloat16;
hop-sum accumulation and the final output stay float32.
"""

import numpy as np


# ---------------------------------------------------------------- config

def make_cfg(N, D, E, H, L, ncores, wlo, cap):
    nsh = N // ncores
    nshp = -(-nsh // 128) * 128          # padded positions per core
    sidxn = -(-nsh // 16) * 16           # scatter num_idxs (16-mult)
    assert nshp - 128 < sidxn <= nshp
    assert wlo <= 32768 and (N + 4) - wlo <= 32768
    assert cap % 128 == 0
    return dict(N=N, D=D, E=E, H=H, L=L, ncores=ncores, nsh=nsh, nshp=nshp,
                sidxn=sidxn, wlo=wlo, cap=cap, s=wlo - 2)


import os as _os

FULL_CFG = make_cfg(N=50000, D=128, E=500000, H=3, L=2, ncores=8,
                    wlo=32768, cap=int(_os.environ.get("CAP", 12288)))


# ----------------------------------------------------- host preprocessing

def build_schedule(scatter_idx, node_idx, cfg):
    """Bucket edges per (core, hop, pass), degree-sort, build rounds.

    Returns:
      chunks: {(h, p): [ncols, ...]} gather chunk widths (cols, <= cap).
      segs:   {(h, p): [[(off, agg_off, width, is_copy), ...] per chunk]}
              DVE segments addressing each gathered chunk.
      gidx:  [ncores, 128, gcols] int16 gather index data (16-wrapped, 8x
             partition-replicated).
      sidx:  [ncores, 128, H*2*scols] int16 scatter index data.
      seg_cols: per-(hop, pass) (col_off, cols) into gidx free dim.
    """
    N, H, nc_, nsh, nshp = cfg["N"], cfg["H"], cfg["ncores"], cfg["nsh"], cfg["nshp"]
    S, wlo, cap, sidxn = cfg["s"], cfg["wlo"], cfg["cap"], cfg["sidxn"]
    zlo = 0                      # xg row 0 is zeros (lo-window pad index)
    zhi = (N + 3) - wlo          # xg row N+3 is zeros, local to hi window

    # per (hop, pass, core): list of per-round idx arrays (window-local)
    rounds_khp = {}
    orders_khp = {}
    maxpref = {}                 # (h, p) -> list of per-round max prefix
    for h in range(H):
        src_h = np.asarray(scatter_idx[h]).astype(np.int64)
        dst_h = np.asarray(node_idx[h]).astype(np.int64)
        core_of = dst_h // nsh
        for k in range(nc_):
            m = core_of == k
            src_k = src_h[m]
            dst_k = dst_h[m] - k * nsh
            for p in range(2):            # 0 = lo, 1 = hi
                if p == 0:
                    mm = src_k < S
                    ps = src_k[mm] + 2            # window-local row index
                else:
                    mm = src_k >= S
                    ps = src_k[mm] + 2 - wlo
                pd = dst_k[mm]
                deg = np.bincount(pd, minlength=nsh)
                order = np.argsort(-deg, kind="stable")
                pos = np.empty(nsh, np.int64)
                pos[order] = np.arange(nsh)
                key = pos[pd]
                so = np.argsort(key, kind="stable")
                ps_s = ps[so]
                key_s = key[so]
                rank = np.arange(len(key_s)) - np.searchsorted(key_s, key_s)
                rlist = []
                maxdeg = int(deg.max()) if len(deg) else 0
                for j in range(maxdeg):
                    rlist.append(ps_s[rank == j].astype(np.int64))
                rounds_khp[(h, p, k)] = rlist
                orders_khp[(h, p, k)] = order
                mp = maxpref.setdefault((h, p), [])
                for j, r in enumerate(rlist):
                    if j < len(mp):
                        mp[j] = max(mp[j], len(r))
                    else:
                        mp.append(len(r))

    # Column layout per (h, p): round 0 padded to nshp (full-width copy,
    # covers deg-0 tails with zero rows), rounds j>=1 padded to the 128-mult
    # of the max prefix over cores, all concatenated, then split into
    # cap-wide gather chunks.  DVE segments carry (chunk-local offset,
    # position offset within the round, width, is_copy).
    chunks = {}
    segs = {}
    npads = {}
    for h in range(H):
        for p in range(2):
            mp = maxpref.get((h, p), [0])
            npad_l = [nshp]
            for j in range(1, len(mp)):
                if mp[j] > 0:
                    npad_l.append(-(-mp[j] // 128) * 128)
            npads[(h, p)] = npad_l
            total = sum(npad_l)
            cl = []
            o = 0
            while o < total:
                cl.append(min(cap, total - o))
                o += cap
            chunks[(h, p)] = cl
            sl = [[] for _ in cl]
            c = 0
            for j, npad in enumerate(npad_l):
                off = 0
                while off < npad:
                    g = c + off
                    ci = g // cap
                    in_off = g % cap
                    w = min(npad - off, cap - in_off)
                    sl[ci].append((in_off, off, w, j == 0))
                    off += w
                c += npad
            segs[(h, p)] = sl

    # gather idx blobs
    seg_cols = {}
    col = 0
    for h in range(H):
        for p in range(2):
            ncols = sum(npads[(h, p)]) // 16
            seg_cols[(h, p)] = (col, ncols)
            col += ncols
    gcols = col
    gidx = np.zeros((nc_, 128, gcols), np.int16)
    for k in range(nc_):
        for h in range(H):
            for p in range(2):
                zpad = zlo if p == 0 else zhi
                rlist = rounds_khp[(h, p, k)]
                npad_l = npads[(h, p)]
                padded = []
                r0 = rlist[0] if rlist else np.zeros(0, np.int64)
                v = np.full(nshp, zpad, np.int64)
                v[: len(r0)] = r0
                padded.append(v)
                for jj, npad in enumerate(npad_l[1:]):
                    j = jj + 1
                    v = np.full(npad, zpad, np.int64)
                    if j < len(rlist):
                        v[: len(rlist[j])] = rlist[j]
                    padded.append(v)
                blob = np.concatenate(padded)
                c0, ncols = seg_cols[(h, p)]
                assert blob.size == ncols * 16, (blob.size, ncols * 16)
                wrapped = blob.reshape(ncols, 16).T.astype(np.int16)  # [16, ncols]
                gidx[k, :, c0:c0 + ncols] = np.tile(wrapped, (8, 1))

    # scatter idx blobs: per (h, p) a column range of width scols
    scols = cfg["sidxn"] // 16
    sidx = np.zeros((nc_, 128, H * 2 * scols), np.int16)
    for k in range(nc_):
        for h in range(H):
            for p in range(2):
                order = orders_khp[(h, p, k)]
                v = np.full(cfg["sidxn"], -1, np.int64)
                v[:nsh] = order
                wrapped = v.reshape(scols, 16).T.astype(np.int16)
                c0 = (h * 2 + p) * scols
                sidx[k, :, c0:c0 + scols] = np.tile(wrapped, (8, 1))

    return chunks, segs, gidx, sidx, seg_cols


# ------------------------------------------------------- device program

def build_program(cfg, chunks, segs, seg_cols, repeat=1, no_collective=False,
                  loop_repeat=None, no_gather=False, shared_xg2=False,
                  tiny_out=False, nqueues=4, sp=False):
    import concourse.bacc as bacc
    import concourse.tile as tile
    from concourse import bass, mybir
    from concourse import library_config

    N, D, H, L = cfg["N"], cfg["D"], cfg["H"], cfg["L"]
    nsh, nshp, sidxn, wlo = cfg["nsh"], cfg["nshp"], cfg["sidxn"], cfg["wlo"]
    nc_cores = cfg["ncores"]
    f32 = mybir.dt.float32
    bf16 = mybir.dt.bfloat16
    i16 = mybir.dt.int16
    CH = nshp // 128                      # position chunks (49 full-size)
    GRP = -(-CH // 2) * 128               # accE/accO free width (25*128)
    scols = sidxn // 16
    gcols = max(c0 + nc for (c0, nc) in seg_cols.values())

    nc = bacc.Bacc("TRN2", target_bir_lowering=False, debug=False,
                   num_devices=cfg["ncores"], num_swdge_queues=nqueues)
    qctr = [0]

    def nextq():
        q = qctr[0] % nqueues
        qctr[0] += 1
        return q

    xg_in = nc.dram_tensor("xg", [N + 4, D], bf16, kind="ExternalInput")
    gidx_t = nc.dram_tensor("gidx", [128, gcols], i16, kind="ExternalInput")
    sidx_t = nc.dram_tensor("sidx", [128, H * 2 * scols], i16, kind="ExternalInput")
    xk_t = nc.dram_tensor("xk", [nshp, D], bf16, kind="ExternalInput")
    identf_t = nc.dram_tensor("identf", [128, 128], f32, kind="ExternalInput")
    identb_t = nc.dram_tensor("identb", [128, 128], bf16, kind="ExternalInput")
    w1_t = nc.dram_tensor("w1f", [L * H * D, D], bf16, kind="ExternalInput")
    w2_t = nc.dram_tensor("w2f", [L * H * D, D], bf16, kind="ExternalInput")
    eps1_t = nc.dram_tensor("eps1", [128, 1], f32, kind="ExternalInput")
    out_t = nc.dram_tensor("out", [128 if tiny_out else nshp, D], f32,
                           kind="ExternalOutput")

    xg2 = nc.dram_tensor("xg2", [N + 4, D], bf16,
                         addr_space="Shared" if shared_xg2 else "Local")
    agin = nc.dram_tensor("agin", [nshp, D], bf16)         # internal AG input

    with tile.TileContext(nc) as tc:
        with (
            tc.tile_pool(name="persist", bufs=1) as pp,
            tc.tile_pool(name="xkpool", bufs=1) as xkp,
            tc.tile_pool(name="big", bufs=2) as bigp,
            tc.tile_pool(name="gt", bufs=2) as gtp,
            tc.tile_pool(name="ix", bufs=1) as ixp,
            tc.tile_pool(name="r1p", bufs=2) as r1p,
            tc.tile_pool(name="ps", bufs=2, space="PSUM") as psp,
            tc.tile_pool(name="ps2", bufs=2, space="PSUM") as ps2p,
        ):
            nc.gpsimd.load_library(library_config.mlp)
            _regs = {}

            def nreg(v):
                if v not in _regs:
                    _regs[v] = nc.gpsimd.to_reg(v)
                return _regs[v]

            identf = pp.tile([128, 128], f32, tag="identf")
            nc.sync.dma_start(identf[:], identf_t[:, :])
            identb = pp.tile([128, 128], bf16, tag="identb")
            nc.sync.dma_start(identb[:], identb_t[:, :])
            eps1 = pp.tile([128, 1], f32, tag="eps1")
            nc.sync.dma_start(eps1[:], eps1_t[:, :])
            sidx_sb = pp.tile([128, H * 2 * scols], i16, tag="sidx")
            nc.sync.dma_start(sidx_sb[:], sidx_t[:, :])
            wtiles = {}
            for l in range(L):
                for h in range(H):
                    wt1 = pp.tile([128, D], bf16, tag=f"w1_{l}_{h}")
                    wt2 = pp.tile([128, D], bf16, tag=f"w2_{l}_{h}")
                    lh = l * H + h
                    nc.sync.dma_start(wt1[:], w1_t[lh * D:(lh + 1) * D, :])
                    nc.sync.dma_start(wt2[:], w2_t[lh * D:(lh + 1) * D, :])
                    wtiles[(l, h)] = (wt1, wt2)

            # zero the pad rows of xg2 (rows 0,1 and N+2,N+3)
            ztile = pp.tile([2, D], bf16, tag="zz")
            nc.vector.memset(ztile[:], 0.0)
            nc.sync.dma_start(xg2[0:2, :], ztile[:])
            nc.sync.dma_start(xg2[N + 2:N + 4, :], ztile[:])

            accE = pp.tile([128, GRP], bf16, tag="accE")
            accO = pp.tile([128, GRP], bf16, tag="accO")
            outaccT = pp.tile([128, nshp], f32, tag="outaccT")
            outf = pp.tile([128, nshp], f32, tag="outf")

            import contextlib
            rep_ctx = (tc.For_i(0, loop_repeat, 1) if loop_repeat
                       else contextlib.nullcontext())
            with rep_ctx:
             for rep in range(repeat):
              xk_sb = xkp.tile([128, nshp], bf16, tag="xk")
              nc.sync.dma_start(
                  xk_sb[:].rearrange("p (c e) -> p c e", e=D),
                  xk_t.ap().rearrange("(c p) e -> p c e", p=128),
              )

              for l in range(L):
                xsrc = xg_in if l == 0 else xg2
                win = {0: xsrc[0:wlo, :], 1: xsrc[wlo:N + 4, :]}
                xkc3 = xk_sb[:].rearrange("p (c e) -> p c e", e=D)

                # outaccT = (1+eps) * x_k^T
                if l == 0:
                    tp = None
                    for c in range(CH):
                        q = c % 4
                        if q == 0:
                            tp = psp.tile([128, 512], bf16, tag="tp")
                        nc.tensor.transpose(
                            tp[:, q * 128:(q + 1) * 128],
                            xk_sb[:, c * 128:(c + 1) * 128], identb[:])
                        if q == 3 or c == CH - 1:
                            w = (q + 1) * 128
                            nc.scalar.activation(
                                outaccT[:, (c - q) * 128:(c - q) * 128 + w],
                                tp[:, :w], mybir.ActivationFunctionType.Copy,
                                scale=eps1[:, 0:1])
                else:
                    # outaccT already holds x^T (pre-back-transpose value):
                    # scale in place.
                    o = 0
                    while o < nshp:
                        w = min(512, nshp - o)
                        nc.scalar.activation(
                            outaccT[:, o:o + w], outaccT[:, o:o + w],
                            mybir.ActivationFunctionType.Copy,
                            scale=eps1[:, 0:1])
                        o += w

                for h in range(H):
                    # canonical accumulators = x_k (even / odd chunks)
                    nE = (CH + 1) // 2
                    nO = CH // 2
                    nc.vector.tensor_copy(
                        accE[:].rearrange("p (c e) -> p c e", e=D)[:, 0:nE, :],
                        xkc3[:, 0:CH:2, :])
                    nc.vector.tensor_copy(
                        accO[:].rearrange("p (c e) -> p c e", e=D)[:, 0:nO, :],
                        xkc3[:, 1:CH:2, :])

                    for p in range(2):
                        c0, ncols = seg_cols[(h, p)]
                        iseg = ixp.tile([128, ncols], i16, tag="iseg")
                        nc.sync.dma_start(iseg[:], gidx_t[:, c0:c0 + ncols])
                        agg = bigp.tile([128, nshp], bf16, tag="big")
                        icol = 0
                        for ci, ncol in enumerate(chunks[(h, p)]):
                            gt = gtp.tile([128, cfg["cap"]], bf16, tag="gt")
                            if no_gather:
                                nc.vector.memset(gt[:, 0:1], 0.0)
                            else:
                                nc.gpsimd.dma_gather(
                                    gt[:, 0:ncol].rearrange(
                                        "p (g e) -> p g e", e=D),
                                    win[p],
                                    iseg[:, icol:icol + ncol // 16],
                                    ncol, nreg(ncol), D, single_packet=sp,
                                    queue_num=nextq())
                            for (off, agg_off, w, is_copy) in segs[(h, p)][ci]:
                                if is_copy:
                                    nc.vector.tensor_copy(
                                        agg[:, agg_off:agg_off + w],
                                        gt[:, off:off + w])
                                else:
                                    nc.vector.tensor_tensor(
                                        agg[:, agg_off:agg_off + w],
                                        agg[:, agg_off:agg_off + w],
                                        gt[:, off:off + w],
                                        mybir.AluOpType.add)
                            icol += ncol // 16
                        sc0 = (h * 2 + p) * scols
                        nc.gpsimd.dma_scatter_add(
                            accE[:].rearrange("p (c e) -> p c e", e=D),
                            agg[:].rearrange("p (c e) -> p c e", e=D),
                            sidx_sb[:, sc0:sc0 + scols],
                            sidxn, nreg(nsh), D,
                            single_packet=False,
                            queue_num=nextq(),
                            sbuf_tokens_per_rank=128,
                            parity_reg=nreg(0),
                            out_ap_other=accO[:].rearrange(
                                "p (c e) -> p c e", e=D))

                    # transpose x+agg into feature-major xpaT
                    xpaT = bigp.tile([128, nshp], bf16, tag="big")
                    tp = None
                    for c in range(CH):
                        q = c % 4
                        if q == 0:
                            tp = psp.tile([128, 512], bf16, tag="tp")
                        buf = accE if c % 2 == 0 else accO
                        g = c // 2
                        nc.tensor.transpose(
                            tp[:, q * 128:(q + 1) * 128],
                            buf[:, g * 128:(g + 1) * 128], identb[:])
                        if q == 3 or c == CH - 1:
                            w = (q + 1) * 128
                            nc.scalar.activation(
                                xpaT[:, (c - q) * 128:(c - q) * 128 + w],
                                tp[:, :w], mybir.ActivationFunctionType.Copy)

                    # MLP: out += relu(xpa @ w1) @ w2   (feature-major)
                    wt1, wt2 = wtiles[(l, h)]
                    o = 0
                    while o < nshp:
                        w = min(512, nshp - o)
                        ps1 = psp.tile([128, 512], f32, tag="mm1")
                        nc.tensor.matmul(ps1[:, :w], wt1[:], xpaT[:, o:o + w],
                                         start=True, stop=True)
                        r1 = r1p.tile([128, 512], bf16, tag="r1")
                        nc.scalar.activation(
                            r1[:, :w], ps1[:, :w],
                            mybir.ActivationFunctionType.Relu)
                        ps2 = ps2p.tile([128, 512], f32, tag="mm2")
                        nc.tensor.matmul(ps2[:, :w], wt2[:], r1[:, :w],
                                         start=True, stop=True)
                        nc.vector.tensor_tensor(
                            outaccT[:, o:o + w], outaccT[:, o:o + w],
                            ps2[:, :w], mybir.AluOpType.add)
                        o += w

                # back-transpose outaccT (f32) -> node-major new x
                if l == 0:
                    xk_new = xkp.tile([128, nshp], bf16, tag="xk")
                    tp = None
                    for c in range(CH):
                        q = c % 4
                        if q == 0:
                            tp = psp.tile([128, 512], f32, tag="tpf")
                        nc.tensor.transpose(
                            tp[:, q * 128:(q + 1) * 128],
                            outaccT[:, c * 128:(c + 1) * 128], identf[:])
                        if q == 3 or c == CH - 1:
                            w = (q + 1) * 128
                            nc.scalar.activation(
                                xk_new[:, (c - q) * 128:(c - q) * 128 + w],
                                tp[:, :w], mybir.ActivationFunctionType.Copy)
                    xk_sb = xk_new
                    nc.sync.dma_start(
                        agin.ap().rearrange("(c p) e -> p c e", p=128),
                        xk_sb[:].rearrange("p (c e) -> p c e", e=D))
                    if no_collective:
                        nc.sync.dma_start(xg2[2:2 + nsh, :], agin[0:nsh, :])
                    else:
                        nc.gpsimd.collective_compute(
                            "AllGather", mybir.AluOpType.bypass,
                            replica_groups=[list(range(nc_cores))],
                            ins=[agin[0:nsh, :]],
                            outs=[xg2[2:2 + N, :]])
                else:
                    tp = None
                    for c in range(CH):
                        q = c % 4
                        if q == 0:
                            tp = psp.tile([128, 512], f32, tag="tpf")
                        nc.tensor.transpose(
                            tp[:, q * 128:(q + 1) * 128],
                            outaccT[:, c * 128:(c + 1) * 128], identf[:])
                        if q == 3 or c == CH - 1:
                            w = (q + 1) * 128
                            nc.scalar.activation(
                                outf[:, (c - q) * 128:(c - q) * 128 + w],
                                tp[:, :w], mybir.ActivationFunctionType.Copy)
                    if tiny_out:
                        nc.sync.dma_start(
                            out_t.ap().rearrange("(c p) e -> p c e", p=128),
                            outf[:, 0:D].rearrange("p (c e) -> p c e", e=D))
                    else:
                        nc.sync.dma_start(
                            out_t.ap().rearrange("(c p) e -> p c e", p=128),
                            outf[:].rearrange("p (c e) -> p c e", e=D))

    nc.compile()
    return nc


# ------------------------------------------------------------- entry

def _prep_inputs(x, w1, w2, eps, scatter_idx, node_idx, cfg):
    import os

    import ml_dtypes
    bf16 = ml_dtypes.bfloat16
    N, D, H, L, nc_ = cfg["N"], cfg["D"], cfg["H"], cfg["L"], cfg["ncores"]
    nsh, nshp = cfg["nsh"], cfg["nshp"]
    x = np.asarray(x, np.float32)
    chunks, segs, gidx, sidx, seg_cols = build_schedule(
        scatter_idx, node_idx, cfg)
    if os.environ.get("ZERO_GIDX"):
        gidx[:] = 0          # same descriptor stream, zero randomness
    if os.environ.get("SEQ_GIDX"):
        # window-local sequential indices: same desc count, fully coalesced
        cols = gidx.shape[2]
        blob = np.arange(cols * 16, dtype=np.int64) % 16384
        w16 = blob.reshape(cols, 16).T.astype(np.int16)
        gidx = np.ascontiguousarray(
            np.broadcast_to(np.tile(w16, (8, 1)), gidx.shape))
    xg = np.zeros((N + 4, D), bf16)
    xg[2:2 + N] = x.astype(bf16)
    w1f = np.asarray(w1, np.float32).reshape(L * H * D, D).astype(bf16)
    w2f = np.asarray(w2, np.float32).reshape(L * H * D, D).astype(bf16)
    eps1 = np.full((128, 1), 1.0 + float(np.asarray(eps).reshape(-1)[0]),
                   np.float32)
    in_maps = []
    for k in range(nc_):
        xk = np.zeros((nshp, D), bf16)
        xk[:nsh] = x[k * nsh:(k + 1) * nsh].astype(bf16)
        in_maps.append({
            "xg": xg, "gidx": gidx[k], "sidx": sidx[k], "xk": xk,
            "w1f": w1f, "w2f": w2f, "eps1": eps1,
            "identf": np.eye(128, dtype=np.float32),
            "identb": np.eye(128).astype(bf16),
        })
    return (chunks, segs, seg_cols), in_maps


def kernel_with_results(x, w1, w2, eps, scatter_idx, node_idx, cfg=None,
                        **run_kwargs):
    cfg = cfg or FULL_CFG
    (chunks, segs, seg_cols), in_maps = _prep_inputs(
        x, w1, w2, eps, scatter_idx, node_idx, cfg)
    nc = build_program(cfg, chunks, segs, seg_cols)

    from concourse.bass_utils import run_bass_kernel_spmd
    res = run_bass_kernel_spmd(nc, in_maps,
                               core_ids=list(range(cfg["ncores"])),
                               **run_kwargs)
    outs = [res.results[k]["out"][:cfg["nsh"]] for k in range(cfg["ncores"])]
    return np.concatenate(outs, axis=0).astype(np.float32), res


def kernel(x, w1, w2, eps, scatter_idx, node_idx):
    out, _ = kernel_with_results(x, w1, w2, eps, scatter_idx, node_idx)
    return out


# revision 19
# speedup vs baseline: 2.3332x; 1.1352x over previous
"""Trainium2 Bass kernel for nn_LocalWLGNN (gnn_message_passing).

Reference computation (per layer l, x: [N, D]):
    out = (1+eps) * x
    for hop in range(H):
        agg = segment_sum(x[scatter_idx[hop]], node_idx[hop], N)
        out += relu((x + agg) @ w1[l,hop]) @ w2[l,hop]
    x = out

Sharding: 8 cores, core k owns destination nodes [k*N/8, (k+1)*N/8).
Each (core, hop) edge list is split by source-node window (lo/hi, so row
indices fit the int16 dma_gather contract), and within each pass the
destinations are sorted by in-degree so that "round j = j-th in-edge of
every destination" covers a contiguous position prefix.  On device:
  - dma_gather (SWDGE) fetches rows from DRAM in large merged chunks
    (rounds concatenated, split only at the chunk capacity),
  - DVE accumulates round segments into a per-pass aggregate (pass order),
  - dma_scatter_add (SBUF parity-split CCE mode) permutes+adds both pass
    aggregates back into canonical order on top of x_k,
  - PE transposes to feature-major, runs the 2-layer MLP,
  - hop outputs accumulate into a feature-major f32 accumulator,
  - an 8-core bf16 AllGather (Shared-output fast path) republishes the
    new node features between layers.
The x-path (features, gather, aggregate, MLP operands) runs in bfloat16;
hop-sum accumulation and the final output stay float32.
"""

import numpy as np


# ---------------------------------------------------------------- config

def make_cfg(N, D, E, H, L, ncores, wlo, cap):
    nsh = N // ncores
    nshp = -(-nsh // 128) * 128          # padded positions per core
    sidxn = -(-nsh // 16) * 16           # scatter num_idxs (16-mult)
    assert nshp - 128 < sidxn <= nshp
    assert wlo <= 32768 and (N + 4) - wlo <= 32768
    assert cap % 128 == 0
    return dict(N=N, D=D, E=E, H=H, L=L, ncores=ncores, nsh=nsh, nshp=nshp,
                sidxn=sidxn, wlo=wlo, cap=cap, s=wlo - 2)


import os as _os

FULL_CFG = make_cfg(N=50000, D=128, E=500000, H=3, L=2, ncores=8,
                    wlo=32768, cap=int(_os.environ.get("CAP", 12288)))


# ----------------------------------------------------- host preprocessing

def build_schedule(scatter_idx, node_idx, cfg):
    """Bucket edges per (core, hop, pass), degree-sort, build rounds.

    Returns:
      chunks: {(h, p): [ncols, ...]} gather chunk widths (cols, <= cap).
      segs:   {(h, p): [[(off, agg_off, width, is_copy), ...] per chunk]}
              DVE segments addressing each gathered chunk.
      gidx:  [ncores, 128, gcols] int16 gather index data (16-wrapped, 8x
             partition-replicated).
      sidx:  [ncores, 128, H*2*scols] int16 scatter index data.
      seg_cols: per-(hop, pass) (col_off, cols) into gidx free dim.
    """
    N, H, nc_, nsh, nshp = cfg["N"], cfg["H"], cfg["ncores"], cfg["nsh"], cfg["nshp"]
    S, wlo, cap, sidxn = cfg["s"], cfg["wlo"], cfg["cap"], cfg["sidxn"]
    zlo = 0                      # xg row 0 is zeros (lo-window pad index)
    zhi = (N + 3) - wlo          # xg row N+3 is zeros, local to hi window

    # per (hop, pass, core): list of per-round idx arrays (window-local)
    rounds_khp = {}
    orders_khp = {}
    maxpref = {}                 # (h, p) -> list of per-round max prefix
    for h in range(H):
        src_h = np.asarray(scatter_idx[h]).astype(np.int64)
        dst_h = np.asarray(node_idx[h]).astype(np.int64)
        core_of = dst_h // nsh
        for k in range(nc_):
            m = core_of == k
            src_k = src_h[m]
            dst_k = dst_h[m] - k * nsh
            for p in range(2):            # 0 = lo, 1 = hi
                if p == 0:
                    mm = src_k < S
                    ps = src_k[mm] + 2            # window-local row index
                else:
                    mm = src_k >= S
                    ps = src_k[mm] + 2 - wlo
                pd = dst_k[mm]
                deg = np.bincount(pd, minlength=nsh)
                order = np.argsort(-deg, kind="stable")
                pos = np.empty(nsh, np.int64)
                pos[order] = np.arange(nsh)
                key = pos[pd]
                so = np.argsort(key, kind="stable")
                ps_s = ps[so]
                key_s = key[so]
                rank = np.arange(len(key_s)) - np.searchsorted(key_s, key_s)
                rlist = []
                maxdeg = int(deg.max()) if len(deg) else 0
                for j in range(maxdeg):
                    rlist.append(ps_s[rank == j].astype(np.int64))
                rounds_khp[(h, p, k)] = rlist
                orders_khp[(h, p, k)] = order
                mp = maxpref.setdefault((h, p), [])
                for j, r in enumerate(rlist):
                    if j < len(mp):
                        mp[j] = max(mp[j], len(r))
                    else:
                        mp.append(len(r))

    # Column layout per (h, p): round 0 padded to nshp (full-width copy,
    # covers deg-0 tails with zero rows), rounds j>=1 padded to the 128-mult
    # of the max prefix over cores, all concatenated, then split into
    # cap-wide gather chunks.  DVE segments carry (chunk-local offset,
    # position offset within the round, width, is_copy).
    chunks = {}
    segs = {}
    npads = {}
    for h in range(H):
        for p in range(2):
            mp = maxpref.get((h, p), [0])
            npad_l = [nshp]
            for j in range(1, len(mp)):
                if mp[j] > 0:
                    npad_l.append(-(-mp[j] // 128) * 128)
            npads[(h, p)] = npad_l
            total = sum(npad_l)
            cl = []
            o = 0
            while o < total:
                cl.append(min(cap, total - o))
                o += cap
            chunks[(h, p)] = cl
            sl = [[] for _ in cl]
            c = 0
            for j, npad in enumerate(npad_l):
                off = 0
                while off < npad:
                    g = c + off
                    ci = g // cap
                    in_off = g % cap
                    w = min(npad - off, cap - in_off)
                    sl[ci].append((in_off, off, w, j == 0))
                    off += w
                c += npad
            segs[(h, p)] = sl

    # gather idx blobs
    seg_cols = {}
    col = 0
    for h in range(H):
        for p in range(2):
            ncols = sum(npads[(h, p)]) // 16
            seg_cols[(h, p)] = (col, ncols)
            col += ncols
    gcols = col
    gidx = np.zeros((nc_, 128, gcols), np.int16)
    for k in range(nc_):
        for h in range(H):
            for p in range(2):
                zpad = zlo if p == 0 else zhi
                rlist = rounds_khp[(h, p, k)]
                npad_l = npads[(h, p)]
                padded = []
                r0 = rlist[0] if rlist else np.zeros(0, np.int64)
                v = np.full(nshp, zpad, np.int64)
                v[: len(r0)] = r0
                padded.append(v)
                for jj, npad in enumerate(npad_l[1:]):
                    j = jj + 1
                    v = np.full(npad, zpad, np.int64)
                    if j < len(rlist):
                        v[: len(rlist[j])] = rlist[j]
                    padded.append(v)
                blob = np.concatenate(padded)
                c0, ncols = seg_cols[(h, p)]
                assert blob.size == ncols * 16, (blob.size, ncols * 16)
                wrapped = blob.reshape(ncols, 16).T.astype(np.int16)  # [16, ncols]
                gidx[k, :, c0:c0 + ncols] = np.tile(wrapped, (8, 1))

    # scatter idx blobs: per (h, p) a column range of width scols
    scols = cfg["sidxn"] // 16
    sidx = np.zeros((nc_, 128, H * 2 * scols), np.int16)
    for k in range(nc_):
        for h in range(H):
            for p in range(2):
                order = orders_khp[(h, p, k)]
                v = np.full(cfg["sidxn"], -1, np.int64)
                v[:nsh] = order
                wrapped = v.reshape(scols, 16).T.astype(np.int16)
                c0 = (h * 2 + p) * scols
                sidx[k, :, c0:c0 + scols] = np.tile(wrapped, (8, 1))

    return chunks, segs, gidx, sidx, seg_cols


# ------------------------------------------------------- device program

def build_program(cfg, chunks, segs, seg_cols, repeat=1, no_collective=False,
                  loop_repeat=None, no_gather=False, shared_xg2=False,
                  tiny_out=False, nqueues=4, sp=False, gbufs=3):
    import concourse.bacc as bacc
    import concourse.tile as tile
    from concourse import bass, mybir
    from concourse import library_config

    N, D, H, L = cfg["N"], cfg["D"], cfg["H"], cfg["L"]
    nsh, nshp, sidxn, wlo = cfg["nsh"], cfg["nshp"], cfg["sidxn"], cfg["wlo"]
    nc_cores = cfg["ncores"]
    f32 = mybir.dt.float32
    bf16 = mybir.dt.bfloat16
    i16 = mybir.dt.int16
    CH = nshp // 128                      # position chunks (49 full-size)
    GRP = -(-CH // 2) * 128               # accE/accO free width (25*128)
    scols = sidxn // 16
    gcols = max(c0 + nc for (c0, nc) in seg_cols.values())

    nc = bacc.Bacc("TRN2", target_bir_lowering=False, debug=False,
                   num_devices=cfg["ncores"], num_swdge_queues=nqueues)
    qctr = [0]

    def nextq():
        q = qctr[0] % nqueues
        qctr[0] += 1
        return q

    xg_in = nc.dram_tensor("xg", [N + 4, D], bf16, kind="ExternalInput")
    gidx_t = nc.dram_tensor("gidx", [128, gcols], i16, kind="ExternalInput")
    sidx_t = nc.dram_tensor("sidx", [128, H * 2 * scols], i16, kind="ExternalInput")
    xk_t = nc.dram_tensor("xk", [nshp, D], bf16, kind="ExternalInput")
    identf_t = nc.dram_tensor("identf", [128, 128], f32, kind="ExternalInput")
    identb_t = nc.dram_tensor("identb", [128, 128], bf16, kind="ExternalInput")
    w1_t = nc.dram_tensor("w1f", [L * H * D, D], bf16, kind="ExternalInput")
    w2_t = nc.dram_tensor("w2f", [L * H * D, D], bf16, kind="ExternalInput")
    eps1_t = nc.dram_tensor("eps1", [128, 1], f32, kind="ExternalInput")
    out_t = nc.dram_tensor("out", [128 if tiny_out else nshp, D], f32,
                           kind="ExternalOutput")

    xg2 = nc.dram_tensor("xg2", [N + 4, D], bf16,
                         addr_space="Shared" if shared_xg2 else "Local")
    agin = nc.dram_tensor("agin", [nshp, D], bf16)         # internal AG input

    with tile.TileContext(nc) as tc:
        with (
            tc.tile_pool(name="persist", bufs=1) as pp,
            tc.tile_pool(name="xkpool", bufs=1) as xkp,
            tc.tile_pool(name="big", bufs=2) as bigp,
            tc.tile_pool(name="gt", bufs=gbufs) as gtp,
            tc.tile_pool(name="ix", bufs=1) as ixp,
            tc.tile_pool(name="r1p", bufs=2) as r1p,
            tc.tile_pool(name="ps", bufs=2, space="PSUM") as psp,
            tc.tile_pool(name="ps2", bufs=2, space="PSUM") as ps2p,
        ):
            nc.gpsimd.load_library(library_config.mlp)
            _regs = {}

            def nreg(v):
                if v not in _regs:
                    _regs[v] = nc.gpsimd.to_reg(v)
                return _regs[v]

            identf = pp.tile([128, 128], f32, tag="identf")
            nc.sync.dma_start(identf[:], identf_t[:, :])
            identb = pp.tile([128, 128], bf16, tag="identb")
            nc.sync.dma_start(identb[:], identb_t[:, :])
            eps1 = pp.tile([128, 1], f32, tag="eps1")
            nc.sync.dma_start(eps1[:], eps1_t[:, :])
            sidx_sb = pp.tile([128, H * 2 * scols], i16, tag="sidx")
            nc.sync.dma_start(sidx_sb[:], sidx_t[:, :])
            wtiles = {}
            for l in range(L):
                for h in range(H):
                    wt1 = pp.tile([128, D], bf16, tag=f"w1_{l}_{h}")
                    wt2 = pp.tile([128, D], bf16, tag=f"w2_{l}_{h}")
                    lh = l * H + h
                    nc.sync.dma_start(wt1[:], w1_t[lh * D:(lh + 1) * D, :])
                    nc.sync.dma_start(wt2[:], w2_t[lh * D:(lh + 1) * D, :])
                    wtiles[(l, h)] = (wt1, wt2)

            # zero the pad rows of xg2 (rows 0,1 and N+2,N+3)
            ztile = pp.tile([2, D], bf16, tag="zz")
            nc.vector.memset(ztile[:], 0.0)
            nc.sync.dma_start(xg2[0:2, :], ztile[:])
            nc.sync.dma_start(xg2[N + 2:N + 4, :], ztile[:])

            accE = pp.tile([128, GRP], bf16, tag="accE")
            accO = pp.tile([128, GRP], bf16, tag="accO")
            outaccT = pp.tile([128, nshp], f32, tag="outaccT")
            outf = pp.tile([128, nshp], f32, tag="outf")

            import contextlib
            rep_ctx = (tc.For_i(0, loop_repeat, 1) if loop_repeat
                       else contextlib.nullcontext())
            with rep_ctx:
             for rep in range(repeat):
              xk_sb = xkp.tile([128, nshp], bf16, tag="xk")
              nc.sync.dma_start(
                  xk_sb[:].rearrange("p (c e) -> p c e", e=D),
                  xk_t.ap().rearrange("(c p) e -> p c e", p=128),
              )

              for l in range(L):
                xsrc = xg_in if l == 0 else xg2
                win = {0: xsrc[0:wlo, :], 1: xsrc[wlo:N + 4, :]}
                xkc3 = xk_sb[:].rearrange("p (c e) -> p c e", e=D)

                # outaccT = (1+eps) * x_k^T
                if l == 0:
                    tp = None
                    for c in range(CH):
                        q = c % 4
                        if q == 0:
                            tp = psp.tile([128, 512], bf16, tag="tp")
                        nc.tensor.transpose(
                            tp[:, q * 128:(q + 1) * 128],
                            xk_sb[:, c * 128:(c + 1) * 128], identb[:])
                        if q == 3 or c == CH - 1:
                            w = (q + 1) * 128
                            nc.scalar.activation(
                                outaccT[:, (c - q) * 128:(c - q) * 128 + w],
                                tp[:, :w], mybir.ActivationFunctionType.Copy,
                                scale=eps1[:, 0:1])
                else:
                    # outaccT already holds x^T (pre-back-transpose value):
                    # scale in place.
                    o = 0
                    while o < nshp:
                        w = min(512, nshp - o)
                        nc.scalar.activation(
                            outaccT[:, o:o + w], outaccT[:, o:o + w],
                            mybir.ActivationFunctionType.Copy,
                            scale=eps1[:, 0:1])
                        o += w

                for h in range(H):
                    # canonical accumulators = x_k (even / odd chunks)
                    nE = (CH + 1) // 2
                    nO = CH // 2
                    nc.vector.tensor_copy(
                        accE[:].rearrange("p (c e) -> p c e", e=D)[:, 0:nE, :],
                        xkc3[:, 0:CH:2, :])
                    nc.vector.tensor_copy(
                        accO[:].rearrange("p (c e) -> p c e", e=D)[:, 0:nO, :],
                        xkc3[:, 1:CH:2, :])

                    for p in range(2):
                        c0, ncols = seg_cols[(h, p)]
                        iseg = ixp.tile([128, ncols], i16, tag="iseg")
                        nc.sync.dma_start(iseg[:], gidx_t[:, c0:c0 + ncols])
                        agg = bigp.tile([128, nshp], bf16, tag="big")
                        icol = 0
                        for ci, ncol in enumerate(chunks[(h, p)]):
                            gt = gtp.tile([128, cfg["cap"]], bf16, tag="gt")
                            if no_gather:
                                nc.vector.memset(gt[:, 0:1], 0.0)
                            else:
                                nc.gpsimd.dma_gather(
                                    gt[:, 0:ncol].rearrange(
                                        "p (g e) -> p g e", e=D),
                                    win[p],
                                    iseg[:, icol:icol + ncol // 16],
                                    ncol, nreg(ncol), D, single_packet=sp,
                                    queue_num=nextq())
                            for (off, agg_off, w, is_copy) in segs[(h, p)][ci]:
                                if is_copy:
                                    nc.vector.tensor_copy(
                                        agg[:, agg_off:agg_off + w],
                                        gt[:, off:off + w])
                                else:
                                    nc.vector.tensor_tensor(
                                        agg[:, agg_off:agg_off + w],
                                        agg[:, agg_off:agg_off + w],
                                        gt[:, off:off + w],
                                        mybir.AluOpType.add)
                            icol += ncol // 16
                        sc0 = (h * 2 + p) * scols
                        nc.gpsimd.dma_scatter_add(
                            accE[:].rearrange("p (c e) -> p c e", e=D),
                            agg[:].rearrange("p (c e) -> p c e", e=D),
                            sidx_sb[:, sc0:sc0 + scols],
                            sidxn, nreg(nsh), D,
                            single_packet=False,
                            queue_num=nextq(),
                            sbuf_tokens_per_rank=128,
                            parity_reg=nreg(0),
                            out_ap_other=accO[:].rearrange(
                                "p (c e) -> p c e", e=D))

                    # transpose x+agg into feature-major xpaT
                    xpaT = bigp.tile([128, nshp], bf16, tag="big")
                    tp = None
                    for c in range(CH):
                        q = c % 4
                        if q == 0:
                            tp = psp.tile([128, 512], bf16, tag="tp")
                        buf = accE if c % 2 == 0 else accO
                        g = c // 2
                        nc.tensor.transpose(
                            tp[:, q * 128:(q + 1) * 128],
                            buf[:, g * 128:(g + 1) * 128], identb[:])
                        if q == 3 or c == CH - 1:
                            w = (q + 1) * 128
                            nc.scalar.activation(
                                xpaT[:, (c - q) * 128:(c - q) * 128 + w],
                                tp[:, :w], mybir.ActivationFunctionType.Copy)

                    # MLP: out += relu(xpa @ w1) @ w2   (feature-major)
                    wt1, wt2 = wtiles[(l, h)]
                    o = 0
                    while o < nshp:
                        w = min(512, nshp - o)
                        ps1 = psp.tile([128, 512], f32, tag="mm1")
                        nc.tensor.matmul(ps1[:, :w], wt1[:], xpaT[:, o:o + w],
                                         start=True, stop=True)
                        r1 = r1p.tile([128, 512], bf16, tag="r1")
                        nc.scalar.activation(
                            r1[:, :w], ps1[:, :w],
                            mybir.ActivationFunctionType.Relu)
                        ps2 = ps2p.tile([128, 512], f32, tag="mm2")
                        nc.tensor.matmul(ps2[:, :w], wt2[:], r1[:, :w],
                                         start=True, stop=True)
                        nc.vector.tensor_tensor(
                            outaccT[:, o:o + w], outaccT[:, o:o + w],
                            ps2[:, :w], mybir.AluOpType.add)
                        o += w

                # back-transpose outaccT (f32) -> node-major new x
                if l == 0:
                    xk_new = xkp.tile([128, nshp], bf16, tag="xk")
                    tp = None
                    for c in range(CH):
                        q = c % 4
                        if q == 0:
                            tp = psp.tile([128, 512], f32, tag="tpf")
                        nc.tensor.transpose(
                            tp[:, q * 128:(q + 1) * 128],
                            outaccT[:, c * 128:(c + 1) * 128], identf[:])
                        if q == 3 or c == CH - 1:
                            w = (q + 1) * 128
                            nc.scalar.activation(
                                xk_new[:, (c - q) * 128:(c - q) * 128 + w],
                                tp[:, :w], mybir.ActivationFunctionType.Copy)
                    xk_sb = xk_new
                    nc.sync.dma_start(
                        agin.ap().rearrange("(c p) e -> p c e", p=128),
                        xk_sb[:].rearrange("p (c e) -> p c e", e=D))
                    if no_collective:
                        nc.sync.dma_start(xg2[2:2 + nsh, :], agin[0:nsh, :])
                    else:
                        nc.gpsimd.collective_compute(
                            "AllGather", mybir.AluOpType.bypass,
                            replica_groups=[list(range(nc_cores))],
                            ins=[agin[0:nsh, :]],
                            outs=[xg2[2:2 + N, :]])
                else:
                    tp = None
                    for c in range(CH):
                        q = c % 4
                        if q == 0:
                            tp = psp.tile([128, 512], f32, tag="tpf")
                        nc.tensor.transpose(
                            tp[:, q * 128:(q + 1) * 128],
                            outaccT[:, c * 128:(c + 1) * 128], identf[:])
                        if q == 3 or c == CH - 1:
                            w = (q + 1) * 128
                            nc.scalar.activation(
                                outf[:, (c - q) * 128:(c - q) * 128 + w],
                                tp[:, :w], mybir.ActivationFunctionType.Copy)
                    if tiny_out:
                        nc.sync.dma_start(
                            out_t.ap().rearrange("(c p) e -> p c e", p=128),
                            outf[:, 0:D].rearrange("p (c e) -> p c e", e=D))
                    else:
                        nc.sync.dma_start(
                            out_t.ap().rearrange("(c p) e -> p c e", p=128),
                            outf[:].rearrange("p (c e) -> p c e", e=D))

    nc.compile()
    return nc


# ------------------------------------------------------------- entry

def _prep_inputs(x, w1, w2, eps, scatter_idx, node_idx, cfg):
    import os

    import ml_dtypes
    bf16 = ml_dtypes.bfloat16
    N, D, H, L, nc_ = cfg["N"], cfg["D"], cfg["H"], cfg["L"], cfg["ncores"]
    nsh, nshp = cfg["nsh"], cfg["nshp"]
    x = np.asarray(x, np.float32)
    chunks, segs, gidx, sidx, seg_cols = build_schedule(
        scatter_idx, node_idx, cfg)
    if os.environ.get("ZERO_GIDX"):
        gidx[:] = 0          # same descriptor stream, zero randomness
    if os.environ.get("SEQ_GIDX"):
        # window-local sequential indices: same desc count, fully coalesced
        cols = gidx.shape[2]
        blob = np.arange(cols * 16, dtype=np.int64) % 16384
        w16 = blob.reshape(cols, 16).T.astype(np.int16)
        gidx = np.ascontiguousarray(
            np.broadcast_to(np.tile(w16, (8, 1)), gidx.shape))
    xg = np.zeros((N + 4, D), bf16)
    xg[2:2 + N] = x.astype(bf16)
    w1f = np.asarray(w1, np.float32).reshape(L * H * D, D).astype(bf16)
    w2f = np.asarray(w2, np.float32).reshape(L * H * D, D).astype(bf16)
    eps1 = np.full((128, 1), 1.0 + float(np.asarray(eps).reshape(-1)[0]),
                   np.float32)
    in_maps = []
    for k in range(nc_):
        xk = np.zeros((nshp, D), bf16)
        xk[:nsh] = x[k * nsh:(k + 1) * nsh].astype(bf16)
        in_maps.append({
            "xg": xg, "gidx": gidx[k], "sidx": sidx[k], "xk": xk,
            "w1f": w1f, "w2f": w2f, "eps1": eps1,
            "identf": np.eye(128, dtype=np.float32),
            "identb": np.eye(128).astype(bf16),
        })
    return (chunks, segs, seg_cols), in_maps


def kernel_with_results(x, w1, w2, eps, scatter_idx, node_idx, cfg=None,
                        **run_kwargs):
    cfg = cfg or FULL_CFG
    (chunks, segs, seg_cols), in_maps = _prep_inputs(
        x, w1, w2, eps, scatter_idx, node_idx, cfg)
    nc = build_program(cfg, chunks, segs, seg_cols)

    from concourse.bass_utils import run_bass_kernel_spmd
    res = run_bass_kernel_spmd(nc, in_maps,
                               core_ids=list(range(cfg["ncores"])),
                               **run_kwargs)
    outs = [res.results[k]["out"][:cfg["nsh"]] for k in range(cfg["ncores"])]
    return np.concatenate(outs, axis=0).astype(np.float32), res


def kernel(x, w1, w2, eps, scatter_idx, node_idx):
    out, _ = kernel_with_results(x, w1, w2, eps, scatter_idx, node_idx)
    return out


# revision 21
# speedup vs baseline: 2.4634x; 1.0558x over previous
"""Trainium2 Bass kernel for nn_LocalWLGNN (gnn_message_passing).

Reference computation (per layer l, x: [N, D]):
    out = (1+eps) * x
    for hop in range(H):
        agg = segment_sum(x[scatter_idx[hop]], node_idx[hop], N)
        out += relu((x + agg) @ w1[l,hop]) @ w2[l,hop]
    x = out

Sharding: 8 cores, core k owns destination nodes [k*N/8, (k+1)*N/8).
Each (core, hop) edge list is split by source-node window (lo/hi, so row
indices fit the int16 dma_gather contract), and within each pass the
destinations are sorted by in-degree so that "round j = j-th in-edge of
every destination" covers a contiguous position prefix.  On device:
  - dma_gather (SWDGE) fetches rows from DRAM in large merged chunks
    (rounds concatenated, split only at the chunk capacity),
  - DVE accumulates round segments into a per-pass aggregate (pass order),
  - dma_scatter_add (SBUF parity-split CCE mode) permutes+adds both pass
    aggregates back into canonical order on top of x_k,
  - PE transposes to feature-major, runs the 2-layer MLP,
  - hop outputs accumulate into a feature-major f32 accumulator,
  - an 8-core bf16 AllGather (Shared-output fast path) republishes the
    new node features between layers.
The x-path (features, gather, aggregate, MLP operands) runs in bfloat16;
hop-sum accumulation and the final output stay float32.
"""

import numpy as np


# ---------------------------------------------------------------- config

def make_cfg(N, D, E, H, L, ncores, wlo, cap):
    nsh = N // ncores
    nshp = -(-nsh // 128) * 128          # padded positions per core
    sidxn = -(-nsh // 16) * 16           # scatter num_idxs (16-mult)
    assert nshp - 128 < sidxn <= nshp
    assert wlo <= 32768 and (N + 4) - wlo <= 32768
    assert cap % 128 == 0
    return dict(N=N, D=D, E=E, H=H, L=L, ncores=ncores, nsh=nsh, nshp=nshp,
                sidxn=sidxn, wlo=wlo, cap=cap, s=wlo - 2)


import os as _os

FULL_CFG = make_cfg(N=50000, D=128, E=500000, H=3, L=2, ncores=8,
                    wlo=32768, cap=int(_os.environ.get("CAP", 12288)))


# ----------------------------------------------------- host preprocessing

def build_schedule(scatter_idx, node_idx, cfg):
    """Bucket edges per (core, hop, pass), degree-sort, build rounds.

    Returns:
      chunks: {(h, p): [ncols, ...]} gather chunk widths (cols, <= cap).
      segs:   {(h, p): [[(off, agg_off, width, is_copy), ...] per chunk]}
              DVE segments addressing each gathered chunk.
      gidx:  [ncores, 128, gcols] int16 gather index data (16-wrapped, 8x
             partition-replicated).
      sidx:  [ncores, 128, H*2*scols] int16 scatter index data.
      seg_cols: per-(hop, pass) (col_off, cols) into gidx free dim.
    """
    N, H, nc_, nsh, nshp = cfg["N"], cfg["H"], cfg["ncores"], cfg["nsh"], cfg["nshp"]
    S, wlo, cap, sidxn = cfg["s"], cfg["wlo"], cfg["cap"], cfg["sidxn"]
    zlo = 0                      # xg row 0 is zeros (lo-window pad index)
    zhi = (N + 3) - wlo          # xg row N+3 is zeros, local to hi window

    # per (hop, pass, core): list of per-round idx arrays (window-local)
    rounds_khp = {}
    orders_khp = {}
    maxpref = {}                 # (h, p) -> list of per-round max prefix
    for h in range(H):
        src_h = np.asarray(scatter_idx[h]).astype(np.int64)
        dst_h = np.asarray(node_idx[h]).astype(np.int64)
        core_of = dst_h // nsh
        for k in range(nc_):
            m = core_of == k
            src_k = src_h[m]
            dst_k = dst_h[m] - k * nsh
            for p in range(2):            # 0 = lo, 1 = hi
                if p == 0:
                    mm = src_k < S
                    ps = src_k[mm] + 2            # window-local row index
                else:
                    mm = src_k >= S
                    ps = src_k[mm] + 2 - wlo
                pd = dst_k[mm]
                deg = np.bincount(pd, minlength=nsh)
                order = np.argsort(-deg, kind="stable")
                pos = np.empty(nsh, np.int64)
                pos[order] = np.arange(nsh)
                key = pos[pd]
                so = np.argsort(key, kind="stable")
                ps_s = ps[so]
                key_s = key[so]
                rank = np.arange(len(key_s)) - np.searchsorted(key_s, key_s)
                rlist = []
                maxdeg = int(deg.max()) if len(deg) else 0
                for j in range(maxdeg):
                    rlist.append(ps_s[rank == j].astype(np.int64))
                rounds_khp[(h, p, k)] = rlist
                orders_khp[(h, p, k)] = order
                mp = maxpref.setdefault((h, p), [])
                for j, r in enumerate(rlist):
                    if j < len(mp):
                        mp[j] = max(mp[j], len(r))
                    else:
                        mp.append(len(r))

    # Column layout per (h, p): round 0 padded to nshp (full-width copy,
    # covers deg-0 tails with zero rows), rounds j>=1 padded to the 128-mult
    # of the max prefix over cores, all concatenated, then split into
    # cap-wide gather chunks.  DVE segments carry (chunk-local offset,
    # position offset within the round, width, is_copy).
    chunks = {}
    segs = {}
    npads = {}
    for h in range(H):
        for p in range(2):
            mp = maxpref.get((h, p), [0])
            npad_l = [nshp]
            for j in range(1, len(mp)):
                if mp[j] > 0:
                    npad_l.append(-(-mp[j] // 128) * 128)
            npads[(h, p)] = npad_l
            total = sum(npad_l)
            cl = []
            o = 0
            while o < total:
                cl.append(min(cap, total - o))
                o += cap
            chunks[(h, p)] = cl
            sl = [[] for _ in cl]
            c = 0
            for j, npad in enumerate(npad_l):
                off = 0
                while off < npad:
                    g = c + off
                    ci = g // cap
                    in_off = g % cap
                    w = min(npad - off, cap - in_off)
                    sl[ci].append((in_off, off, w, j == 0))
                    off += w
                c += npad
            segs[(h, p)] = sl

    # gather idx blobs
    seg_cols = {}
    col = 0
    for h in range(H):
        for p in range(2):
            ncols = sum(npads[(h, p)]) // 16
            seg_cols[(h, p)] = (col, ncols)
            col += ncols
    gcols = col
    gidx = np.zeros((nc_, 128, gcols), np.int16)
    for k in range(nc_):
        for h in range(H):
            for p in range(2):
                zpad = zlo if p == 0 else zhi
                rlist = rounds_khp[(h, p, k)]
                npad_l = npads[(h, p)]
                padded = []
                r0 = rlist[0] if rlist else np.zeros(0, np.int64)
                v = np.full(nshp, zpad, np.int64)
                v[: len(r0)] = r0
                padded.append(v)
                for jj, npad in enumerate(npad_l[1:]):
                    j = jj + 1
                    v = np.full(npad, zpad, np.int64)
                    if j < len(rlist):
                        v[: len(rlist[j])] = rlist[j]
                    padded.append(v)
                blob = np.concatenate(padded)
                c0, ncols = seg_cols[(h, p)]
                assert blob.size == ncols * 16, (blob.size, ncols * 16)
                wrapped = blob.reshape(ncols, 16).T.astype(np.int16)  # [16, ncols]
                gidx[k, :, c0:c0 + ncols] = np.tile(wrapped, (8, 1))

    # scatter idx blobs: per (h, p) a column range of width scols
    scols = cfg["sidxn"] // 16
    sidx = np.zeros((nc_, 128, H * 2 * scols), np.int16)
    for k in range(nc_):
        for h in range(H):
            for p in range(2):
                order = orders_khp[(h, p, k)]
                v = np.full(cfg["sidxn"], -1, np.int64)
                v[:nsh] = order
                wrapped = v.reshape(scols, 16).T.astype(np.int16)
                c0 = (h * 2 + p) * scols
                sidx[k, :, c0:c0 + scols] = np.tile(wrapped, (8, 1))

    return chunks, segs, gidx, sidx, seg_cols


# ------------------------------------------------------- device program

def build_program(cfg, chunks, segs, seg_cols, repeat=1, no_collective=False,
                  loop_repeat=None, no_gather=False, shared_xg2=False,
                  tiny_out=False, nqueues=4, sp=False, gbufs=3):
    import concourse.bacc as bacc
    import concourse.tile as tile
    from concourse import bass, mybir
    from concourse import library_config

    N, D, H, L = cfg["N"], cfg["D"], cfg["H"], cfg["L"]
    nsh, nshp, sidxn, wlo = cfg["nsh"], cfg["nshp"], cfg["sidxn"], cfg["wlo"]
    nc_cores = cfg["ncores"]
    f32 = mybir.dt.float32
    bf16 = mybir.dt.bfloat16
    i16 = mybir.dt.int16
    CH = nshp // 128                      # position chunks (49 full-size)
    GRP = -(-CH // 2) * 128               # accE/accO free width (25*128)
    scols = sidxn // 16
    gcols = max(c0 + nc for (c0, nc) in seg_cols.values())

    nc = bacc.Bacc("TRN2", target_bir_lowering=False, debug=False,
                   num_devices=cfg["ncores"], num_swdge_queues=nqueues)
    qctr = [0]

    def nextq():
        q = qctr[0] % nqueues
        qctr[0] += 1
        return q

    xg_in = nc.dram_tensor("xg", [N + 4, D], bf16, kind="ExternalInput")
    gidx_t = nc.dram_tensor("gidx", [128, gcols], i16, kind="ExternalInput")
    sidx_t = nc.dram_tensor("sidx", [128, H * 2 * scols], i16, kind="ExternalInput")
    xk_t = nc.dram_tensor("xk", [nshp, D], bf16, kind="ExternalInput")
    identf_t = nc.dram_tensor("identf", [128, 128], f32, kind="ExternalInput")
    identb_t = nc.dram_tensor("identb", [128, 128], bf16, kind="ExternalInput")
    w1_t = nc.dram_tensor("w1f", [L * H * D, D], bf16, kind="ExternalInput")
    w2_t = nc.dram_tensor("w2f", [L * H * D, D], bf16, kind="ExternalInput")
    eps1_t = nc.dram_tensor("eps1", [128, 1], f32, kind="ExternalInput")
    out_t = nc.dram_tensor("out", [128 if tiny_out else nshp, D], f32,
                           kind="ExternalOutput")

    xg2 = nc.dram_tensor("xg2", [N + 4, D], bf16,
                         addr_space="Shared" if shared_xg2 else "Local")
    agin = nc.dram_tensor("agin", [nshp, D], bf16)         # internal AG input

    with tile.TileContext(nc) as tc:
        with (
            tc.tile_pool(name="persist", bufs=1) as pp,
            tc.tile_pool(name="xkpool", bufs=1) as xkp,
            tc.tile_pool(name="big", bufs=3) as bigp,
            tc.tile_pool(name="gt", bufs=gbufs) as gtp,
            tc.tile_pool(name="r1p", bufs=2) as r1p,
            tc.tile_pool(name="ps", bufs=2, space="PSUM") as psp,
            tc.tile_pool(name="ps2", bufs=2, space="PSUM") as ps2p,
        ):
            nc.gpsimd.load_library(library_config.mlp)
            _regs = {}

            def nreg(v):
                if v not in _regs:
                    _regs[v] = nc.gpsimd.to_reg(v)
                return _regs[v]

            identf = pp.tile([128, 128], f32, tag="identf")
            nc.sync.dma_start(identf[:], identf_t[:, :])
            identb = pp.tile([128, 128], bf16, tag="identb")
            nc.sync.dma_start(identb[:], identb_t[:, :])
            eps1 = pp.tile([128, 1], f32, tag="eps1")
            nc.sync.dma_start(eps1[:], eps1_t[:, :])
            sidx_sb = pp.tile([128, H * 2 * scols], i16, tag="sidx")
            nc.sync.dma_start(sidx_sb[:], sidx_t[:, :])
            gall = pp.tile([128, gcols], i16, tag="gall")
            nc.sync.dma_start(gall[:], gidx_t[:, :])
            wtiles = {}
            for l in range(L):
                for h in range(H):
                    wt1 = pp.tile([128, D], bf16, tag=f"w1_{l}_{h}")
                    wt2 = pp.tile([128, D], bf16, tag=f"w2_{l}_{h}")
                    lh = l * H + h
                    nc.sync.dma_start(wt1[:], w1_t[lh * D:(lh + 1) * D, :])
                    nc.sync.dma_start(wt2[:], w2_t[lh * D:(lh + 1) * D, :])
                    wtiles[(l, h)] = (wt1, wt2)

            # zero the pad rows of xg2 (rows 0,1 and N+2,N+3)
            ztile = pp.tile([2, D], bf16, tag="zz")
            nc.vector.memset(ztile[:], 0.0)
            nc.sync.dma_start(xg2[0:2, :], ztile[:])
            nc.sync.dma_start(xg2[N + 2:N + 4, :], ztile[:])

            accE = pp.tile([128, GRP], bf16, tag="accE")
            accO = pp.tile([128, GRP], bf16, tag="accO")
            outaccT = pp.tile([128, nshp], f32, tag="outaccT")

            import contextlib
            rep_ctx = (tc.For_i(0, loop_repeat, 1) if loop_repeat
                       else contextlib.nullcontext())
            with rep_ctx:
             for rep in range(repeat):
              xk_sb = xkp.tile([128, nshp], bf16, tag="xk")
              nc.sync.dma_start(
                  xk_sb[:].rearrange("p (c e) -> p c e", e=D),
                  xk_t.ap().rearrange("(c p) e -> p c e", p=128),
              )

              for l in range(L):
                xsrc = xg_in if l == 0 else xg2
                win = {0: xsrc[0:wlo, :], 1: xsrc[wlo:N + 4, :]}
                xkc3 = xk_sb[:].rearrange("p (c e) -> p c e", e=D)

                # outaccT = (1+eps) * x_k^T
                if l == 0:
                    tp = None
                    for c in range(CH):
                        q = c % 4
                        if q == 0:
                            tp = psp.tile([128, 512], bf16, tag="tp")
                        nc.tensor.transpose(
                            tp[:, q * 128:(q + 1) * 128],
                            xk_sb[:, c * 128:(c + 1) * 128], identb[:])
                        if q == 3 or c == CH - 1:
                            w = (q + 1) * 128
                            nc.scalar.activation(
                                outaccT[:, (c - q) * 128:(c - q) * 128 + w],
                                tp[:, :w], mybir.ActivationFunctionType.Copy,
                                scale=eps1[:, 0:1])
                else:
                    # outaccT already holds x^T (pre-back-transpose value):
                    # scale in place.
                    o = 0
                    while o < nshp:
                        w = min(512, nshp - o)
                        nc.scalar.activation(
                            outaccT[:, o:o + w], outaccT[:, o:o + w],
                            mybir.ActivationFunctionType.Copy,
                            scale=eps1[:, 0:1])
                        o += w

                for h in range(H):
                    # canonical accumulators = x_k (even / odd chunks)
                    nE = (CH + 1) // 2
                    nO = CH // 2
                    nc.vector.tensor_copy(
                        accE[:].rearrange("p (c e) -> p c e", e=D)[:, 0:nE, :],
                        xkc3[:, 0:CH:2, :])
                    nc.vector.tensor_copy(
                        accO[:].rearrange("p (c e) -> p c e", e=D)[:, 0:nO, :],
                        xkc3[:, 1:CH:2, :])

                    for p in range(2):
                        c0, ncols = seg_cols[(h, p)]
                        iseg = gall[:, c0:c0 + ncols]
                        agg = bigp.tile([128, nshp], bf16, tag="big")
                        icol = 0
                        for ci, ncol in enumerate(chunks[(h, p)]):
                            gt = gtp.tile([128, cfg["cap"]], bf16, tag="gt")
                            if no_gather:
                                nc.vector.memset(gt[:, 0:1], 0.0)
                            else:
                                nc.gpsimd.dma_gather(
                                    gt[:, 0:ncol].rearrange(
                                        "p (g e) -> p g e", e=D),
                                    win[p],
                                    iseg[:, icol:icol + ncol // 16],
                                    ncol, nreg(ncol), D, single_packet=sp,
                                    queue_num=nextq())
                            for (off, agg_off, w, is_copy) in segs[(h, p)][ci]:
                                if is_copy:
                                    nc.vector.tensor_copy(
                                        agg[:, agg_off:agg_off + w],
                                        gt[:, off:off + w])
                                else:
                                    nc.vector.tensor_tensor(
                                        agg[:, agg_off:agg_off + w],
                                        agg[:, agg_off:agg_off + w],
                                        gt[:, off:off + w],
                                        mybir.AluOpType.add)
                            icol += ncol // 16
                        sc0 = (h * 2 + p) * scols
                        nc.gpsimd.dma_scatter_add(
                            accE[:].rearrange("p (c e) -> p c e", e=D),
                            agg[:].rearrange("p (c e) -> p c e", e=D),
                            sidx_sb[:, sc0:sc0 + scols],
                            sidxn, nreg(nsh), D,
                            single_packet=False,
                            queue_num=nextq(),
                            sbuf_tokens_per_rank=128,
                            parity_reg=nreg(0),
                            out_ap_other=accO[:].rearrange(
                                "p (c e) -> p c e", e=D))

                    # transpose x+agg into feature-major xpaT
                    xpaT = bigp.tile([128, nshp], bf16, tag="big")
                    tp = None
                    for c in range(CH):
                        q = c % 4
                        if q == 0:
                            tp = psp.tile([128, 512], bf16, tag="tp")
                        buf = accE if c % 2 == 0 else accO
                        g = c // 2
                        nc.tensor.transpose(
                            tp[:, q * 128:(q + 1) * 128],
                            buf[:, g * 128:(g + 1) * 128], identb[:])
                        if q == 3 or c == CH - 1:
                            w = (q + 1) * 128
                            nc.scalar.activation(
                                xpaT[:, (c - q) * 128:(c - q) * 128 + w],
                                tp[:, :w], mybir.ActivationFunctionType.Copy)

                    # MLP: out += relu(xpa @ w1) @ w2   (feature-major)
                    wt1, wt2 = wtiles[(l, h)]
                    o = 0
                    while o < nshp:
                        w = min(512, nshp - o)
                        ps1 = psp.tile([128, 512], f32, tag="mm1")
                        nc.tensor.matmul(ps1[:, :w], wt1[:], xpaT[:, o:o + w],
                                         start=True, stop=True)
                        r1 = r1p.tile([128, 512], bf16, tag="r1")
                        nc.scalar.activation(
                            r1[:, :w], ps1[:, :w],
                            mybir.ActivationFunctionType.Relu)
                        ps2 = ps2p.tile([128, 512], f32, tag="mm2")
                        nc.tensor.matmul(ps2[:, :w], wt2[:], r1[:, :w],
                                         start=True, stop=True)
                        nc.vector.tensor_tensor(
                            outaccT[:, o:o + w], outaccT[:, o:o + w],
                            ps2[:, :w], mybir.AluOpType.add)
                        o += w

                # back-transpose outaccT (f32) -> node-major new x
                if l == 0:
                    xk_new = xkp.tile([128, nshp], bf16, tag="xk")
                    tp = None
                    for c in range(CH):
                        q = c % 4
                        if q == 0:
                            tp = psp.tile([128, 512], f32, tag="tpf")
                        nc.tensor.transpose(
                            tp[:, q * 128:(q + 1) * 128],
                            outaccT[:, c * 128:(c + 1) * 128], identf[:])
                        if q == 3 or c == CH - 1:
                            w = (q + 1) * 128
                            nc.scalar.activation(
                                xk_new[:, (c - q) * 128:(c - q) * 128 + w],
                                tp[:, :w], mybir.ActivationFunctionType.Copy)
                    xk_sb = xk_new
                    nc.sync.dma_start(
                        agin.ap().rearrange("(c p) e -> p c e", p=128),
                        xk_sb[:].rearrange("p (c e) -> p c e", e=D))
                    if no_collective:
                        nc.sync.dma_start(xg2[2:2 + nsh, :], agin[0:nsh, :])
                    else:
                        nc.gpsimd.collective_compute(
                            "AllGather", mybir.AluOpType.bypass,
                            replica_groups=[list(range(nc_cores))],
                            ins=[agin[0:nsh, :]],
                            outs=[xg2[2:2 + N, :]])
                else:
                    tp = None
                    for c in range(CH):
                        q = c % 4
                        if q == 0:
                            tp = psp.tile([128, 512], f32, tag="tpf")
                        nc.tensor.transpose(
                            tp[:, q * 128:(q + 1) * 128],
                            outaccT[:, c * 128:(c + 1) * 128], identf[:])
                        if q == 3 or c == CH - 1:
                            w = (q + 1) * 128
                            ost = r1p.tile([128, 512], f32, tag="ost")
                            nc.scalar.activation(
                                ost[:, :w], tp[:, :w],
                                mybir.ActivationFunctionType.Copy)
                            c0 = c - q
                            if tiny_out:
                                if c0 == 0:
                                    nc.sync.dma_start(
                                        out_t.ap().rearrange(
                                            "(c p) e -> p c e", p=128),
                                        ost[:, 0:D].rearrange(
                                            "p (c e) -> p c e", e=D))
                            else:
                                nc.sync.dma_start(
                                    out_t.ap().rearrange(
                                        "(c p) e -> p c e",
                                        p=128)[:, c0:c0 + w // 128, :],
                                    ost[:, :w].rearrange(
                                        "p (c e) -> p c e", e=D))

    nc.compile()
    return nc


# ------------------------------------------------------------- entry

def _prep_inputs(x, w1, w2, eps, scatter_idx, node_idx, cfg):
    import os

    import ml_dtypes
    bf16 = ml_dtypes.bfloat16
    N, D, H, L, nc_ = cfg["N"], cfg["D"], cfg["H"], cfg["L"], cfg["ncores"]
    nsh, nshp = cfg["nsh"], cfg["nshp"]
    x = np.asarray(x, np.float32)
    chunks, segs, gidx, sidx, seg_cols = build_schedule(
        scatter_idx, node_idx, cfg)
    if os.environ.get("ZERO_GIDX"):
        gidx[:] = 0          # same descriptor stream, zero randomness
    if os.environ.get("SEQ_GIDX"):
        # window-local sequential indices: same desc count, fully coalesced
        cols = gidx.shape[2]
        blob = np.arange(cols * 16, dtype=np.int64) % 16384
        w16 = blob.reshape(cols, 16).T.astype(np.int16)
        gidx = np.ascontiguousarray(
            np.broadcast_to(np.tile(w16, (8, 1)), gidx.shape))
    xg = np.zeros((N + 4, D), bf16)
    xg[2:2 + N] = x.astype(bf16)
    w1f = np.asarray(w1, np.float32).reshape(L * H * D, D).astype(bf16)
    w2f = np.asarray(w2, np.float32).reshape(L * H * D, D).astype(bf16)
    eps1 = np.full((128, 1), 1.0 + float(np.asarray(eps).reshape(-1)[0]),
                   np.float32)
    in_maps = []
    for k in range(nc_):
        xk = np.zeros((nshp, D), bf16)
        xk[:nsh] = x[k * nsh:(k + 1) * nsh].astype(bf16)
        in_maps.append({
            "xg": xg, "gidx": gidx[k], "sidx": sidx[k], "xk": xk,
            "w1f": w1f, "w2f": w2f, "eps1": eps1,
            "identf": np.eye(128, dtype=np.float32),
            "identb": np.eye(128).astype(bf16),
        })
    return (chunks, segs, seg_cols), in_maps


def kernel_with_results(x, w1, w2, eps, scatter_idx, node_idx, cfg=None,
                        **run_kwargs):
    cfg = cfg or FULL_CFG
    (chunks, segs, seg_cols), in_maps = _prep_inputs(
        x, w1, w2, eps, scatter_idx, node_idx, cfg)
    nc = build_program(cfg, chunks, segs, seg_cols)

    from concourse.bass_utils import run_bass_kernel_spmd
    res = run_bass_kernel_spmd(nc, in_maps,
                               core_ids=list(range(cfg["ncores"])),
                               **run_kwargs)
    outs = [res.results[k]["out"][:cfg["nsh"]] for k in range(cfg["ncores"])]
    return np.concatenate(outs, axis=0).astype(np.float32), res


def kernel(x, w1, w2, eps, scatter_idx, node_idx):
    out, _ = kernel_with_results(x, w1, w2, eps, scatter_idx, node_idx)
    return out
